# revision 18
# baseline (speedup 1.0000x reference)
"""BasesDecomposition GNN message passing on 8 Trainium2 NeuronCores.

Math (reference):
    seg  = edge_type * N + target
    h    = segment_sum(x[source] * ew, seg)        # (R, N, D)
    out  = einsum('rb,bio,rni->no', bw, bases, h)  # (N, D)

Restructuring: fold the basis decomposition into per-relation weight
matrices on the host:  W_r = sum_b bw[r,b] * bases[b]  (R, D, D).
Then  out[n] = sum_r h_r[n] @ W_r  with  h_r = segment_sum over edges of
relation r.  Per-edge work no longer involves the basis dimension, so the
device inner loop is:

    per 128-edge tile (single relation r):
        oh[e,m]     = (iota[m] == tgt_e) * ew_e        (1 DVE op, 128 wide)
        psum_h_r   += xg_tile^T @ oh                   (PE, accumulate)
    per 4-relation group: hsb = copy(psum_h)           (1 DVE op)
    per relation:  psum_out += hsb_r^T @ W_r           (PE, accumulate)

Sharding: nodes by target-id range across the 8 cores (no collective);
each core consumes only edges targeting its node range.  Edges sorted by
(node-tile, src-half, relation); each (tile, half, relation) run is padded
to 128-edge tiles with uniform caps (max over cores/tiles) so one SPMD
program serves all cores.  Gathers of x rows (bf16, split in lo/hi tables
for int16 indices) run on all 4 SWDGE queues (Q7 core pairs) round-robin.
"""

import numpy as np

import concourse.bass as bass
import concourse.mybir as mybir
import concourse.tile as tile
from concourse import bacc
from concourse.bass_utils import run_bass_kernel_spmd
from concourse.tile import add_dep_helper

NCORES = 8
P = 128          # edges per tile (matmul contraction dim)
M = 128          # nodes per node-tile (selector block width)
SPLIT = 32768    # x row split so gather indices fit int16
CAST_CHUNK = 1024  # x rows cast per prologue step
GRP = 4          # relations per PSUM bank group

TRACE = False
LAST_PROFILE = None

_PROG_CACHE = {}


def _build_program(N, D, R, NPC, NT, caps_lo, caps_hi):
    fp = mybir.dt.float32
    bf = mybir.dt.bfloat16
    i16 = mybir.dt.int16
    NHI = N - SPLIT

    T_LO = int(sum(caps_lo))
    T_HI = int(sum(caps_hi))
    T = T_LO + T_HI
    NGRP = (R + GRP - 1) // GRP

    # Layout: lo tiles rel-major (tile index = position in xg / mf), then hi
    # tiles rel-major.  Processing order is rel-consecutive (all tiles of rel
    # r back-to-back) so each PSUM region's accumulation group is sequential.
    lo_tiles = {}
    off = 0
    for r in range(R):
        lo_tiles[r] = list(range(off, off + caps_lo[r]))
        off += caps_lo[r]
    hi_tiles = {}
    for r in range(R):
        hi_tiles[r] = list(range(off, off + caps_hi[r]))
        off += caps_hi[r]
    active_rels = [r for r in range(R) if caps_lo[r] + caps_hi[r] > 0]
    sched = []  # (tile_index, rel, is_first_of_rel, is_last_of_rel)
    for r in active_rels:
        tiles_r = lo_tiles[r] + hi_tiles[r]
        for j, t in enumerate(tiles_r):
            sched.append((t, r, j == 0, j == len(tiles_r) - 1))
    # group g is complete after this position in sched
    grp_done = {}
    for g in range(NGRP):
        rels = [r for r in active_rels if g * GRP <= r < (g + 1) * GRP]
        if rels:
            last_r = max(rels)
            for k, (t, r, fi, la) in enumerate(sched):
                if r == last_r and la:
                    grp_done[k] = g

    nc = bacc.Bacc(
        "TRN2",
        target_bir_lowering=False,
        debug=False,
        num_devices=NCORES,
        num_swdge_queues=4,
    )
    x_d = nc.dram_tensor("x", [N, D], fp, kind="ExternalInput").ap()
    wr_d = nc.dram_tensor("wr16", [R, D, D], bf, kind="ExternalInput").ap()
    iota_d = nc.dram_tensor("iota", [P, M], bf, kind="ExternalInput").ap()
    idx_d = nc.dram_tensor("idx16", [NT, P, T * 8], i16, kind="ExternalInput").ap()
    # meta per tile t: cols 4t..4t+3 = (tgt, -tgt, -ew, ew)
    mf_d = nc.dram_tensor("meta_f", [NT, P, 4 * T], fp, kind="ExternalInput").ap()
    out_d = nc.dram_tensor("out", [NPC, D], fp, kind="ExternalOutput").ap()

    # tiles whose one-hot build runs on the Scalar (ACT) engine
    act_tiles = set(k for k in range(T) if k % 7 >= 5)

    xlo_d = nc.dram_tensor("xlo", [SPLIT, D], bf).ap()
    xhi_d = nc.dram_tensor("xhi", [NHI, D], bf).ap()

    with tile.TileContext(nc) as tc:
        with (
            tc.tile_pool(name="const", bufs=1) as constp,
            tc.tile_pool(name="castp", bufs=3) as castp,
            tc.tile_pool(name="meta", bufs=3) as metap,
            tc.tile_pool(name="xg", bufs=2) as xgp,
            tc.tile_pool(name="sel", bufs=6) as selp,
            tc.tile_pool(name="hsb", bufs=2) as hsbp,
            tc.tile_pool(name="osb", bufs=3) as osbp,
            tc.tile_pool(name="psh", bufs=1, space="PSUM") as pshp,
            tc.tile_pool(name="pso", bufs=2, space="PSUM") as psop,
        ):
            iota_sb = constp.tile([P, M], bf)
            nc.sync.dma_start(out=iota_sb[:], in_=iota_d[:])
            wr_sb = constp.tile([P, R * D], bf)
            for r in range(R):
                nc.sync.dma_start(out=wr_sb[:, r * D:(r + 1) * D], in_=wr_d[r])

            # ---- prologue: cast x (f32) into xlo/xhi (bf16) in DRAM ----
            cast_dmas = []
            for r0 in range(0, N, CAST_CHUNK):
                rows = min(CAST_CHUNK, N - r0)
                q = rows // P
                rem = rows - q * P
                src = x_d[r0:r0 + q * P, :].rearrange("(q p) f -> p q f", p=P)
                tf = castp.tile([P, q * D], fp, tag="cast_f")
                nc.sync.dma_start(out=tf[:].rearrange("p (q f) -> p q f", f=D), in_=src)
                tb = castp.tile([P, q * D], bf, tag="cast_b")
                nc.scalar.activation(
                    out=tb[:], in_=tf[:], func=mybir.ActivationFunctionType.Copy
                )
                if r0 < SPLIT:
                    dst = xlo_d[r0:r0 + q * P, :]
                else:
                    dst = xhi_d[r0 - SPLIT:r0 - SPLIT + q * P, :]
                d = nc.sync.dma_start(
                    out=dst.rearrange("(q p) f -> p q f", p=P),
                    in_=tb[:].rearrange("p (q f) -> p q f", f=D),
                )
                cast_dmas.append(d)
                if rem:
                    r1 = r0 + q * P
                    tf2 = castp.tile([P, D], fp, tag="cast_f2")
                    nc.sync.dma_start(out=tf2[:rem, :], in_=x_d[r1:r1 + rem, :])
                    tb2 = castp.tile([P, D], bf, tag="cast_b2")
                    nc.vector.tensor_copy(out=tb2[:rem, :], in_=tf2[:rem, :])
                    d2 = nc.sync.dma_start(
                        out=xhi_d[r1 - SPLIT:r1 - SPLIT + rem, :], in_=tb2[:rem, :]
                    )
                    cast_dmas.append(d2)
            fencet = constp.tile([P, 1], fp)
            fence = nc.gpsimd.memset(fencet[:], 0.0)
            for d in cast_dmas:
                add_dep_helper(fence.ins, d.ins, reason="x-cast fence")

            for nt in range(NT):
                m_lo = nt * M
                m_sz = min(M, NPC - m_lo)

                idxt = metap.tile([P, T * 8], i16, tag="idx")
                mf = metap.tile([P, 4 * T], fp, tag="mf")
                nc.sync.dma_start(out=idxt[:], in_=idx_d[nt])
                nc.sync.dma_start(out=mf[:], in_=mf_d[nt])

                xg = xgp.tile([P, T * D], bf, tag="xg")
                gathers = []
                GMAX = 8  # 1024 idxs/call: SWDGE desc ring is ~16KB/lane
                qrr = nt  # rotate queue usage across node tiles
                for t0 in range(0, T_LO, GMAX):
                    tn = min(GMAX, T_LO - t0)
                    gathers.append(nc.gpsimd.dma_gather(
                        out_ap=xg[:, t0 * D:(t0 + tn) * D].rearrange(
                            "p (t f) -> p t f", f=D),
                        in_ap=xlo_d[:],
                        idxs_ap=idxt[:, t0 * 8:(t0 + tn) * 8],
                        num_idxs=P * tn,
                        num_idxs_reg=P * tn,
                        elem_size=D,
                        queue_num=qrr % 4,
                    ))
                    qrr += 1
                for t0 in range(T_LO, T, GMAX):
                    tn = min(GMAX, T - t0)
                    gathers.append(nc.gpsimd.dma_gather(
                        out_ap=xg[:, t0 * D:(t0 + tn) * D].rearrange(
                            "p (t f) -> p t f", f=D),
                        in_ap=xhi_d[:],
                        idxs_ap=idxt[:, t0 * 8:(t0 + tn) * 8],
                        num_idxs=P * tn,
                        num_idxs_reg=P * tn,
                        elem_size=D,
                        queue_num=qrr % 4,
                    ))
                    qrr += 1
                for g in gathers:
                    add_dep_helper(g.ins, fence.ins, reason="gather after x cast")

                psh = [pshp.tile([P, GRP * M], fp, tag=f"psh{g}", name=f"psh{g}")
                       for g in range(NGRP)]
                pso = psop.tile([P, D], fp)
                hsb = {}
                for k, (t, r, is_first, is_last) in enumerate(sched):
                    g = r // GRP
                    rloc = r - g * GRP
                    oh = selp.tile([P, M], bf, tag="oh")
                    if k in act_tiles:
                        # on ACT: oh = relu(ew - ew*|iota - tgt|)  (exact 0/1
                        # one-hot times ew since iota/tgt are small ints)
                        ab = selp.tile([P, M], bf, tag="oha")
                        nc.scalar.activation(
                            out=ab[:], in_=iota_sb[:],
                            func=mybir.ActivationFunctionType.Abs,
                            bias=mf[:, 4 * t + 1:4 * t + 2],
                        )
                        nc.scalar.activation(
                            out=oh[:], in_=ab[:],
                            func=mybir.ActivationFunctionType.Relu,
                            bias=mf[:, 4 * t + 3:4 * t + 4],
                            scale=mf[:, 4 * t + 2:4 * t + 3],
                        )
                    else:
                        nc.vector.tensor_scalar(
                            oh[:],
                            iota_sb[:],
                            mf[:, 4 * t:4 * t + 1],
                            mf[:, 4 * t + 3:4 * t + 4],
                            mybir.AluOpType.is_equal,
                            mybir.AluOpType.mult,
                        )
                    nc.tensor.matmul(
                        out=psh[g][:, rloc * M:(rloc + 1) * M],
                        lhsT=xg[:, t * D:(t + 1) * D],
                        rhs=oh[:],
                        start=is_first,
                        stop=is_last,
                    )
                    if k in grp_done:
                        gd = grp_done[k]
                        ht = hsbp.tile([P, GRP * M], bf, tag=f"hsb{gd}")
                        nc.scalar.activation(
                            out=ht[:], in_=psh[gd][:],
                            func=mybir.ActivationFunctionType.Copy,
                        )
                        hsb[gd] = ht

                for i, r in enumerate(active_rels):
                    g = r // GRP
                    rloc = r - g * GRP
                    nc.tensor.matmul(
                        out=pso[:m_sz, :],
                        lhsT=hsb[g][:, rloc * M:rloc * M + m_sz],
                        rhs=wr_sb[:, r * D:(r + 1) * D],
                        start=(i == 0),
                        stop=(i == len(active_rels) - 1),
                    )
                osb = osbp.tile([P, D], fp)
                nc.scalar.activation(
                    out=osb[:m_sz, :], in_=pso[:m_sz, :],
                    func=mybir.ActivationFunctionType.Copy,
                )
                nc.sync.dma_start(out=out_d[m_lo:m_lo + m_sz, :], in_=osb[:m_sz, :])
    nc.compile()
    return nc


def _wrap16(a):
    """Pack flat index array (n,) into dma_gather layout (128, n/16):
    index j lives at [j % 16, j // 16]; rows replicated to 128."""
    n = a.shape[0]
    w = a.reshape(n // 16, 16).T  # (16, n/16)
    return np.tile(w, (8, 1))


def kernel(x, source, target, edge_type, edge_weights, base_weights, bases):
    global LAST_PROFILE
    import ml_dtypes

    x = np.ascontiguousarray(np.asarray(x), dtype=np.float32)
    src = np.asarray(source).astype(np.int64)
    tgt = np.asarray(target).astype(np.int64)
    et = np.asarray(edge_type).astype(np.int64)
    ew = np.ascontiguousarray(np.asarray(edge_weights), dtype=np.float32)
    bw = np.ascontiguousarray(np.asarray(base_weights), dtype=np.float32)
    bs = np.ascontiguousarray(np.asarray(bases), dtype=np.float32)

    N, D = x.shape
    R, B = bw.shape
    E = src.shape[0]
    NPC = N // NCORES
    NT = (NPC + M - 1) // M

    # fold basis decomposition: W_r = sum_b bw[r,b] * bases[b]
    wr = np.einsum("rb,bio->rio", bw, bs).astype(ml_dtypes.bfloat16)

    # ---- host-side sharding: sort by (node-tile, src-half, relation) ----
    hi = (src >= SPLIT).astype(np.int64)
    core = tgt // NPC
    local = tgt - core * NPC
    ntile = local // M
    ntg = core * NT + ntile
    # src as final key: gather reads walk HBM monotonically within each run
    order = np.lexsort((src, et, hi, ntg))
    src_s = src[order]
    et_s = et[order]
    ew_s = ew[order]
    hi_s = hi[order]
    ntg_s = ntg[order]
    tgtf_s = (local[order] - (ntg_s % NT) * M).astype(np.float32)

    # per (core, nt, half, rel) counts -> uniform caps (max over core, nt)
    cell = (ntg_s * 2 + hi_s) * R + et_s  # monotone under the sort
    counts = np.bincount(cell, minlength=NCORES * NT * 2 * R)
    cnt4 = counts.reshape(NCORES * NT, 2, R)
    caps_lo = [int(np.ceil(cnt4[:, 0, r].max() / P)) for r in range(R)]
    caps_hi = [int(np.ceil(cnt4[:, 1, r].max() / P)) for r in range(R)]
    T_LO = sum(caps_lo)
    T_HI = sum(caps_hi)
    T = T_LO + T_HI

    # slot base of each (half, rel) run within a node tile
    run_base = np.zeros((2, R), dtype=np.int64)
    off = 0
    for r in range(R):
        run_base[0, r] = off
        off += caps_lo[r] * P
    for r in range(R):
        run_base[1, r] = off
        off += caps_hi[r] * P
    slots_per_nt = off  # == T * P

    starts = np.zeros(NCORES * NT * 2 * R + 1, dtype=np.int64)
    np.cumsum(counts, out=starts[1:])
    pos = np.arange(E, dtype=np.int64) - starts[cell]
    slot = ntg_s * slots_per_nt + run_base[hi_s, et_s] + pos

    nslots = NCORES * NT * slots_per_nt
    idx_flat = np.zeros(nslots, dtype=np.int16)
    tg_flat = np.zeros(nslots, dtype=np.float32)
    ew_flat = np.zeros(nslots, dtype=np.float32)
    idx_flat[slot] = (src_s - hi_s * SPLIT).astype(np.int16)
    tg_flat[slot] = tgtf_s
    ew_flat[slot] = ew_s

    # pad slots repeat the last real index of their (nt, half) section so the
    # gather's HBM reads stay monotone / page-local (ew=0 kills their output)
    real_pos = np.full(nslots, -1, dtype=np.int64)
    real_pos[slot] = slot
    np.maximum.accumulate(real_pos, out=real_pos)
    inslot = np.arange(nslots, dtype=np.int64) % slots_per_nt
    sec_start = (np.arange(nslots, dtype=np.int64) - inslot) + np.where(
        inslot < T_LO * P, 0, T_LO * P
    )
    fill_ok = real_pos >= sec_start
    idx_flat = np.where(fill_ok, idx_flat[np.maximum(real_pos, 0)], idx_flat)

    # dma_gather wrapped index layout per node tile: lo call then hi call
    idx16 = np.empty((NCORES, NT, P, T * 8), dtype=np.int16)
    idx_nt = idx_flat.reshape(NCORES, NT, slots_per_nt)
    for c in range(NCORES):
        for nt in range(NT):
            if T_LO:
                idx16[c, nt, :, :T_LO * 8] = _wrap16(idx_nt[c, nt, :T_LO * P])
            if T_HI:
                idx16[c, nt, :, T_LO * 8:] = _wrap16(idx_nt[c, nt, T_LO * P:])

    # meta_f: (C, NT, P, 4T) with [p, 4t..4t+3] = (tgt, -tgt, -ew, ew)
    tgr = tg_flat.reshape(NCORES, NT, T, P)
    ewr = ew_flat.reshape(NCORES, NT, T, P)
    mf5 = np.stack([tgr, -tgr, -ewr, ewr], axis=-1)  # (C, NT, T, P, 4)
    meta_f = np.ascontiguousarray(mf5.transpose(0, 1, 3, 2, 4)).reshape(
        NCORES, NT, P, 4 * T
    )

    iota_arr = np.ascontiguousarray(
        np.broadcast_to(np.arange(M, dtype=ml_dtypes.bfloat16), (P, M))
    )

    key = (N, D, R, NPC, NT, tuple(caps_lo), tuple(caps_hi))
    if key not in _PROG_CACHE:
        _PROG_CACHE[key] = _build_program(N, D, R, NPC, NT, caps_lo, caps_hi)
    nc = _PROG_CACHE[key]

    in_maps = [
        dict(
            x=x,
            wr16=wr,
            iota=iota_arr,
            idx16=idx16[c],
            meta_f=meta_f[c],
        )
        for c in range(NCORES)
    ]
    res = run_bass_kernel_spmd(nc, in_maps, list(range(NCORES)), trace=TRACE)
    LAST_PROFILE = res
    out = np.concatenate([res.results[c]["out"] for c in range(NCORES)], axis=0)
    return out


# revision 24
# speedup vs baseline: 1.0215x; 1.0215x over previous
"""BasesDecomposition GNN message passing on 8 Trainium2 NeuronCores.

Math (reference):
    seg  = edge_type * N + target
    h    = segment_sum(x[source] * ew, seg)        # (R, N, D)
    out  = einsum('rb,bio,rni->no', bw, bases, h)  # (N, D)

Restructuring: fold the basis decomposition into per-relation weight
matrices on the host:  W_r = sum_b bw[r,b] * bases[b]  (R, D, D).
Then  out[n] = sum_r h_r[n] @ W_r  with  h_r = segment_sum over edges of
relation r.  Per-edge work no longer involves the basis dimension, so the
device inner loop is:

    per 128-edge tile (single relation r):
        oh[e,m]     = (iota[m] == tgt_e) * ew_e        (1 DVE op, 128 wide)
        psum_h_r   += xg_tile^T @ oh                   (PE, accumulate)
    per 4-relation group: hsb = copy(psum_h)           (1 DVE op)
    per relation:  psum_out += hsb_r^T @ W_r           (PE, accumulate)

Sharding: nodes by target-id range across the 8 cores (no collective);
each core consumes only edges targeting its node range.  Edges sorted by
(node-tile, src-half, relation); each (tile, half, relation) run is padded
to 128-edge tiles with uniform caps (max over cores/tiles) so one SPMD
program serves all cores.  Gathers of x rows (bf16, split in lo/hi tables
for int16 indices) run on all 4 SWDGE queues (Q7 core pairs) round-robin.
"""

import numpy as np

import concourse.bass as bass
import concourse.mybir as mybir
import concourse.tile as tile
from concourse import bacc
from concourse.bass_utils import run_bass_kernel_spmd
from concourse.tile import add_dep_helper

NCORES = 8
P = 128          # edges per tile (matmul contraction dim)
M = 128          # nodes per node-tile (selector block width)
SPLIT = 32768    # x row split so gather indices fit int16
CAST_CHUNK = 1024  # x rows cast per prologue step
GRP = 4          # relations per PSUM bank group

TRACE = False
LAST_PROFILE = None

_PROG_CACHE = {}


def _build_program(N, D, R, NPC, NT, caps_lo, caps_hi):
    fp = mybir.dt.float32
    bf = mybir.dt.bfloat16
    i16 = mybir.dt.int16
    NHI = N - SPLIT

    T_LO = int(sum(caps_lo))
    T_HI = int(sum(caps_hi))
    T = T_LO + T_HI
    NGRP = (R + GRP - 1) // GRP

    # Layout: lo tiles rel-major (tile index = position in xg / mf), then hi
    # tiles rel-major.  Processing order is rel-consecutive (all tiles of rel
    # r back-to-back) so each PSUM region's accumulation group is sequential.
    lo_tiles = {}
    off = 0
    for r in range(R):
        lo_tiles[r] = list(range(off, off + caps_lo[r]))
        off += caps_lo[r]
    hi_tiles = {}
    for r in range(R):
        hi_tiles[r] = list(range(off, off + caps_hi[r]))
        off += caps_hi[r]
    active_rels = [r for r in range(R) if caps_lo[r] + caps_hi[r] > 0]
    sched = []  # (tile_index, rel, is_first_of_rel, is_last_of_rel)
    for r in active_rels:
        tiles_r = lo_tiles[r] + hi_tiles[r]
        for j, t in enumerate(tiles_r):
            sched.append((t, r, j == 0, j == len(tiles_r) - 1))
    # group g is complete after this position in sched
    grp_done = {}
    for g in range(NGRP):
        rels = [r for r in active_rels if g * GRP <= r < (g + 1) * GRP]
        if rels:
            last_r = max(rels)
            for k, (t, r, fi, la) in enumerate(sched):
                if r == last_r and la:
                    grp_done[k] = g

    nc = bacc.Bacc(
        "TRN2",
        target_bir_lowering=False,
        debug=False,
        num_devices=NCORES,
        num_swdge_queues=4,
    )
    x_d = nc.dram_tensor("x", [N, D], fp, kind="ExternalInput").ap()
    wr_d = nc.dram_tensor("wr16", [R, D, D], bf, kind="ExternalInput").ap()
    iota_d = nc.dram_tensor("iota", [P, M], bf, kind="ExternalInput").ap()
    idx_d = nc.dram_tensor("idx16", [NT, P, T * 8], i16, kind="ExternalInput").ap()
    # meta per tile t: cols 2t, 2t+1 = (tgt, ew)
    mf_d = nc.dram_tensor("meta_f", [NT, P, 2 * T], fp, kind="ExternalInput").ap()
    out_d = nc.dram_tensor("out", [NPC, D], fp, kind="ExternalOutput").ap()

    xlo_d = nc.dram_tensor("xlo", [SPLIT, D], bf).ap()
    xhi_d = nc.dram_tensor("xhi", [NHI, D], bf).ap()

    with tile.TileContext(nc) as tc:
        with (
            tc.tile_pool(name="const", bufs=1) as constp,
            tc.tile_pool(name="castp", bufs=3) as castp,
            tc.tile_pool(name="meta", bufs=3) as metap,
            tc.tile_pool(name="xg", bufs=2) as xgp,
            tc.tile_pool(name="sel", bufs=6) as selp,
            tc.tile_pool(name="hsb", bufs=2) as hsbp,
            tc.tile_pool(name="osb", bufs=3) as osbp,
            tc.tile_pool(name="psh", bufs=1, space="PSUM") as pshp,
            tc.tile_pool(name="pso", bufs=2, space="PSUM") as psop,
        ):
            iota_sb = constp.tile([P, M], bf)
            nc.sync.dma_start(out=iota_sb[:], in_=iota_d[:])
            wr_sb = constp.tile([P, R * D], bf)
            for r in range(R):
                nc.sync.dma_start(out=wr_sb[:, r * D:(r + 1) * D], in_=wr_d[r])

            # ---- prologue: cast x (f32) into xlo/xhi (bf16) in DRAM ----
            cast_dmas = []
            for r0 in range(0, N, CAST_CHUNK):
                rows = min(CAST_CHUNK, N - r0)
                q = rows // P
                rem = rows - q * P
                src = x_d[r0:r0 + q * P, :].rearrange("(q p) f -> p q f", p=P)
                tf = castp.tile([P, q * D], fp, tag="cast_f")
                nc.sync.dma_start(out=tf[:].rearrange("p (q f) -> p q f", f=D), in_=src)
                tb = castp.tile([P, q * D], bf, tag="cast_b")
                nc.scalar.activation(
                    out=tb[:], in_=tf[:], func=mybir.ActivationFunctionType.Copy
                )
                if r0 < SPLIT:
                    dst = xlo_d[r0:r0 + q * P, :]
                else:
                    dst = xhi_d[r0 - SPLIT:r0 - SPLIT + q * P, :]
                d = nc.sync.dma_start(
                    out=dst.rearrange("(q p) f -> p q f", p=P),
                    in_=tb[:].rearrange("p (q f) -> p q f", f=D),
                )
                cast_dmas.append(d)
                if rem:
                    r1 = r0 + q * P
                    tf2 = castp.tile([P, D], fp, tag="cast_f2")
                    nc.sync.dma_start(out=tf2[:rem, :], in_=x_d[r1:r1 + rem, :])
                    tb2 = castp.tile([P, D], bf, tag="cast_b2")
                    nc.vector.tensor_copy(out=tb2[:rem, :], in_=tf2[:rem, :])
                    d2 = nc.sync.dma_start(
                        out=xhi_d[r1 - SPLIT:r1 - SPLIT + rem, :], in_=tb2[:rem, :]
                    )
                    cast_dmas.append(d2)
            fencet = constp.tile([P, 1], fp)
            fence = nc.gpsimd.memset(fencet[:], 0.0)
            for d in cast_dmas:
                add_dep_helper(fence.ins, d.ins, reason="x-cast fence")

            for nt in range(NT):
                m_lo = nt * M
                m_sz = min(M, NPC - m_lo)

                idxt = metap.tile([P, T * 8], i16, tag="idx")
                mf = metap.tile([P, 2 * T], fp, tag="mf")
                nc.sync.dma_start(out=idxt[:], in_=idx_d[nt])
                nc.sync.dma_start(out=mf[:], in_=mf_d[nt])

                xg = xgp.tile([P, T * D], bf, tag="xg")
                gathers = []
                GMAX = 8  # 1024 idxs/call: larger overflows SWDGE desc ring
                qrr = nt  # rotate queue usage across node tiles
                for t0 in range(0, T_LO, GMAX):
                    tn = min(GMAX, T_LO - t0)
                    gathers.append(nc.gpsimd.dma_gather(
                        out_ap=xg[:, t0 * D:(t0 + tn) * D].rearrange(
                            "p (t f) -> p t f", f=D),
                        in_ap=xlo_d[:],
                        idxs_ap=idxt[:, t0 * 8:(t0 + tn) * 8],
                        num_idxs=P * tn,
                        num_idxs_reg=P * tn,
                        elem_size=D,
                        queue_num=qrr % 4,
                    ))
                    qrr += 1
                for t0 in range(T_LO, T, GMAX):
                    tn = min(GMAX, T - t0)
                    gathers.append(nc.gpsimd.dma_gather(
                        out_ap=xg[:, t0 * D:(t0 + tn) * D].rearrange(
                            "p (t f) -> p t f", f=D),
                        in_ap=xhi_d[:],
                        idxs_ap=idxt[:, t0 * 8:(t0 + tn) * 8],
                        num_idxs=P * tn,
                        num_idxs_reg=P * tn,
                        elem_size=D,
                        queue_num=qrr % 4,
                    ))
                    qrr += 1
                for g in gathers:
                    add_dep_helper(g.ins, fence.ins, reason="gather after x cast")

                psh = [pshp.tile([P, GRP * M], fp, tag=f"psh{g}", name=f"psh{g}")
                       for g in range(NGRP)]
                pso = psop.tile([P, D], fp)
                hsb = {}
                for k, (t, r, is_first, is_last) in enumerate(sched):
                    g = r // GRP
                    rloc = r - g * GRP
                    oh = selp.tile([P, M], bf, tag="oh")
                    nc.vector.tensor_scalar(
                        oh[:],
                        iota_sb[:],
                        mf[:, 2 * t:2 * t + 1],
                        mf[:, 2 * t + 1:2 * t + 2],
                        mybir.AluOpType.is_equal,
                        mybir.AluOpType.mult,
                    )
                    nc.tensor.matmul(
                        out=psh[g][:, rloc * M:(rloc + 1) * M],
                        lhsT=xg[:, t * D:(t + 1) * D],
                        rhs=oh[:],
                        start=is_first,
                        stop=is_last,
                    )
                    if k in grp_done:
                        gd = grp_done[k]
                        ht = hsbp.tile([P, GRP * M], bf, tag=f"hsb{gd}")
                        nc.scalar.activation(
                            out=ht[:], in_=psh[gd][:],
                            func=mybir.ActivationFunctionType.Copy,
                        )
                        hsb[gd] = ht

                for i, r in enumerate(active_rels):
                    g = r // GRP
                    rloc = r - g * GRP
                    nc.tensor.matmul(
                        out=pso[:m_sz, :],
                        lhsT=hsb[g][:, rloc * M:rloc * M + m_sz],
                        rhs=wr_sb[:, r * D:(r + 1) * D],
                        start=(i == 0),
                        stop=(i == len(active_rels) - 1),
                    )
                osb = osbp.tile([P, D], fp)
                nc.scalar.activation(
                    out=osb[:m_sz, :], in_=pso[:m_sz, :],
                    func=mybir.ActivationFunctionType.Copy,
                )
                nc.sync.dma_start(out=out_d[m_lo:m_lo + m_sz, :], in_=osb[:m_sz, :])
    nc.compile()
    return nc


def _wrap16(a):
    """Pack flat index array (n,) into dma_gather layout (128, n/16):
    index j lives at [j % 16, j // 16]; rows replicated to 128."""
    n = a.shape[0]
    w = a.reshape(n // 16, 16).T  # (16, n/16)
    return np.tile(w, (8, 1))


def kernel(x, source, target, edge_type, edge_weights, base_weights, bases):
    global LAST_PROFILE
    import ml_dtypes

    x = np.ascontiguousarray(np.asarray(x), dtype=np.float32)
    src = np.asarray(source).astype(np.int64)
    tgt = np.asarray(target).astype(np.int64)
    et = np.asarray(edge_type).astype(np.int64)
    ew = np.ascontiguousarray(np.asarray(edge_weights), dtype=np.float32)
    bw = np.ascontiguousarray(np.asarray(base_weights), dtype=np.float32)
    bs = np.ascontiguousarray(np.asarray(bases), dtype=np.float32)

    N, D = x.shape
    R, B = bw.shape
    E = src.shape[0]
    NPC = N // NCORES
    NT = (NPC + M - 1) // M

    # fold basis decomposition: W_r = sum_b bw[r,b] * bases[b]
    wr = np.einsum("rb,bio->rio", bw, bs).astype(ml_dtypes.bfloat16)

    # ---- host-side sharding: sort by (node-tile, src-half, relation) ----
    hi = (src >= SPLIT).astype(np.int64)
    core = tgt // NPC
    local = tgt - core * NPC
    ntile = local // M
    ntg = core * NT + ntile
    # src as final key: gather reads walk HBM monotonically within each run
    order = np.lexsort((src, et, hi, ntg))
    src_s = src[order]
    et_s = et[order]
    ew_s = ew[order]
    hi_s = hi[order]
    ntg_s = ntg[order]
    tgtf_s = (local[order] - (ntg_s % NT) * M).astype(np.float32)

    # per (core, nt, half, rel) counts -> uniform caps (max over core, nt)
    cell = (ntg_s * 2 + hi_s) * R + et_s  # monotone under the sort
    counts = np.bincount(cell, minlength=NCORES * NT * 2 * R)
    cnt4 = counts.reshape(NCORES * NT, 2, R)
    caps_lo = [int(np.ceil(cnt4[:, 0, r].max() / P)) for r in range(R)]
    caps_hi = [int(np.ceil(cnt4[:, 1, r].max() / P)) for r in range(R)]
    T_LO = sum(caps_lo)
    T_HI = sum(caps_hi)
    T = T_LO + T_HI

    # slot base of each (half, rel) run within a node tile
    run_base = np.zeros((2, R), dtype=np.int64)
    off = 0
    for r in range(R):
        run_base[0, r] = off
        off += caps_lo[r] * P
    for r in range(R):
        run_base[1, r] = off
        off += caps_hi[r] * P
    slots_per_nt = off  # == T * P

    starts = np.zeros(NCORES * NT * 2 * R + 1, dtype=np.int64)
    np.cumsum(counts, out=starts[1:])
    pos = np.arange(E, dtype=np.int64) - starts[cell]
    slot = ntg_s * slots_per_nt + run_base[hi_s, et_s] + pos

    nslots = NCORES * NT * slots_per_nt
    idx_flat = np.zeros(nslots, dtype=np.int16)
    tg_flat = np.zeros(nslots, dtype=np.float32)
    ew_flat = np.zeros(nslots, dtype=np.float32)
    idx_flat[slot] = (src_s - hi_s * SPLIT).astype(np.int16)
    tg_flat[slot] = tgtf_s
    ew_flat[slot] = ew_s

    # pad slots repeat the last real index of their (nt, half) section so the
    # gather's HBM reads stay monotone / page-local (ew=0 kills their output)
    real_pos = np.full(nslots, -1, dtype=np.int64)
    real_pos[slot] = slot
    np.maximum.accumulate(real_pos, out=real_pos)
    inslot = np.arange(nslots, dtype=np.int64) % slots_per_nt
    sec_start = (np.arange(nslots, dtype=np.int64) - inslot) + np.where(
        inslot < T_LO * P, 0, T_LO * P
    )
    fill_ok = real_pos >= sec_start
    idx_flat = np.where(fill_ok, idx_flat[np.maximum(real_pos, 0)], idx_flat)

    # dma_gather wrapped index layout per node tile: lo call then hi call
    idx16 = np.empty((NCORES, NT, P, T * 8), dtype=np.int16)
    idx_nt = idx_flat.reshape(NCORES, NT, slots_per_nt)
    for c in range(NCORES):
        for nt in range(NT):
            if T_LO:
                idx16[c, nt, :, :T_LO * 8] = _wrap16(idx_nt[c, nt, :T_LO * P])
            if T_HI:
                idx16[c, nt, :, T_LO * 8:] = _wrap16(idx_nt[c, nt, T_LO * P:])

    # meta_f: (C, NT, P, 4T) with [p, 4t..4t+3] = (tgt, -tgt, -ew, ew)
    tgr = tg_flat.reshape(NCORES, NT, T, P)
    ewr = ew_flat.reshape(NCORES, NT, T, P)
    mf5 = np.stack([tgr, ewr], axis=-1)  # (C, NT, T, P, 2)
    meta_f = np.ascontiguousarray(mf5.transpose(0, 1, 3, 2, 4)).reshape(
        NCORES, NT, P, 2 * T
    )

    iota_arr = np.ascontiguousarray(
        np.broadcast_to(np.arange(M, dtype=ml_dtypes.bfloat16), (P, M))
    )

    key = (N, D, R, NPC, NT, tuple(caps_lo), tuple(caps_hi))
    if key not in _PROG_CACHE:
        _PROG_CACHE[key] = _build_program(N, D, R, NPC, NT, caps_lo, caps_hi)
    nc = _PROG_CACHE[key]

    in_maps = [
        dict(
            x=x,
            wr16=wr,
            iota=iota_arr,
            idx16=idx16[c],
            meta_f=meta_f[c],
        )
        for c in range(NCORES)
    ]
    res = run_bass_kernel_spmd(nc, in_maps, list(range(NCORES)), trace=TRACE)
    LAST_PROFILE = res
    out = np.concatenate([res.results[c]["out"] for c in range(NCORES)], axis=0)
    return out


# revision 26
# speedup vs baseline: 1.1098x; 1.0865x over previous
"""BasesDecomposition GNN message passing on 8 Trainium2 NeuronCores.

Math (reference):
    seg  = edge_type * N + target
    h    = segment_sum(x[source] * ew, seg)        # (R, N, D)
    out  = einsum('rb,bio,rni->no', bw, bases, h)  # (N, D)

Restructuring: fold the basis decomposition into per-relation weight
matrices on the host:  W_r = sum_b bw[r,b] * bases[b]  (R, D, D).
Then  out[n] = sum_r h_r[n] @ W_r  with  h_r = segment_sum over edges of
relation r.  Per-edge work no longer involves the basis dimension, so the
device inner loop is:

    per 128-edge tile (single relation r):
        oh[e,m]     = (iota[m] == tgt_e) * ew_e        (1 DVE op, 128 wide)
        psum_h_r   += xg_tile^T @ oh                   (PE, accumulate)
    per 4-relation group: hsb = copy(psum_h)           (1 DVE op)
    per relation:  psum_out += hsb_r^T @ W_r           (PE, accumulate)

Sharding: nodes by target-id range across the 8 cores (no collective);
each core consumes only edges targeting its node range.  Edges sorted by
(node-tile, src-half, relation); each (tile, half, relation) run is padded
to 128-edge tiles with uniform caps (max over cores/tiles) so one SPMD
program serves all cores.  Gathers of x rows (bf16, split in lo/hi tables
for int16 indices) run on all 4 SWDGE queues (Q7 core pairs) round-robin.
"""

import numpy as np

import concourse.bass as bass
import concourse.mybir as mybir
import concourse.tile as tile
from concourse import bacc
from concourse.bass_utils import run_bass_kernel_spmd
from concourse.tile import add_dep_helper

NCORES = 8
P = 128          # edges per tile (matmul contraction dim)
M = 128          # nodes per node-tile (selector block width)
SPLIT = 32768    # x row split so gather indices fit int16
CAST_CHUNK = 1024  # x rows cast per prologue step
GRP = 4          # relations per PSUM bank group

TRACE = False
LAST_PROFILE = None

_PROG_CACHE = {}


def _build_program(N, D, R, NPC, NT, caps_lo, caps_hi):
    fp = mybir.dt.float32
    bf = mybir.dt.bfloat16
    i16 = mybir.dt.int16
    NHI = N - SPLIT

    T_LO = int(sum(caps_lo))
    T_HI = int(sum(caps_hi))
    T = T_LO + T_HI
    NGRP = (R + GRP - 1) // GRP

    # Layout: lo tiles rel-major (tile index = position in xg / mf), then hi
    # tiles rel-major.  Processing order is rel-consecutive (all tiles of rel
    # r back-to-back) so each PSUM region's accumulation group is sequential.
    lo_tiles = {}
    off = 0
    for r in range(R):
        lo_tiles[r] = list(range(off, off + caps_lo[r]))
        off += caps_lo[r]
    hi_tiles = {}
    for r in range(R):
        hi_tiles[r] = list(range(off, off + caps_hi[r]))
        off += caps_hi[r]
    active_rels = [r for r in range(R) if caps_lo[r] + caps_hi[r] > 0]
    sched = []  # (tile_index, rel, is_first_of_rel, is_last_of_rel)
    for r in active_rels:
        tiles_r = lo_tiles[r] + hi_tiles[r]
        for j, t in enumerate(tiles_r):
            sched.append((t, r, j == 0, j == len(tiles_r) - 1))
    # group g is complete after this position in sched
    grp_done = {}
    for g in range(NGRP):
        rels = [r for r in active_rels if g * GRP <= r < (g + 1) * GRP]
        if rels:
            last_r = max(rels)
            for k, (t, r, fi, la) in enumerate(sched):
                if r == last_r and la:
                    grp_done[k] = g

    nc = bacc.Bacc(
        "TRN2",
        target_bir_lowering=False,
        debug=False,
        num_devices=NCORES,
        num_swdge_queues=4,
    )
    x_d = nc.dram_tensor("x", [N, D], fp, kind="ExternalInput").ap()
    wr_d = nc.dram_tensor("wr16", [R, D, D], bf, kind="ExternalInput").ap()
    iota_d = nc.dram_tensor("iota", [P, M], bf, kind="ExternalInput").ap()
    idx_d = nc.dram_tensor("idx16", [NT, P, T * 8], i16, kind="ExternalInput").ap()
    # meta per tile t: cols 2t, 2t+1 = (tgt, ew)
    mf_d = nc.dram_tensor("meta_f", [NT, P, 2 * T], fp, kind="ExternalInput").ap()
    out_d = nc.dram_tensor("out", [NPC, D], fp, kind="ExternalOutput").ap()

    xlo_d = nc.dram_tensor("xlo", [SPLIT, D], bf).ap()
    xhi_d = nc.dram_tensor("xhi", [NHI, D], bf).ap()

    with tile.TileContext(nc) as tc:
        with (
            tc.tile_pool(name="const", bufs=1) as constp,
            tc.tile_pool(name="meta", bufs=3) as metap,
            tc.tile_pool(name="xg", bufs=2) as xgp,
            tc.tile_pool(name="sel", bufs=56) as selp,
            tc.tile_pool(name="hsb", bufs=2) as hsbp,
            tc.tile_pool(name="osb", bufs=3) as osbp,
            tc.tile_pool(name="psh", bufs=1, space="PSUM") as pshp,
            tc.tile_pool(name="pso", bufs=2, space="PSUM") as psop,
        ):
            iota_sb = constp.tile([P, M], bf)
            nc.sync.dma_start(out=iota_sb[:], in_=iota_d[:])
            wr_sb = constp.tile([P, R * D], bf)
            for r in range(R):
                nc.sync.dma_start(out=wr_sb[:, r * D:(r + 1) * D], in_=wr_d[r])

            # ---- prologue: bulk SWDGE cast x (f32) -> xlo/xhi (bf16), DRAM
            # to DRAM with dtype convert; no SBUF bounce ----
            cast_dmas = [
                nc.gpsimd.dma_start(out=xlo_d[:], in_=x_d[:SPLIT, :]),
                nc.gpsimd.dma_start(out=xhi_d[:], in_=x_d[SPLIT:, :]),
            ]
            fencet = constp.tile([P, 1], fp)
            fence = nc.gpsimd.memset(fencet[:], 0.0)
            for d in cast_dmas:
                add_dep_helper(fence.ins, d.ins, reason="x-cast fence")

            for nt in range(NT):
                m_lo = nt * M
                m_sz = min(M, NPC - m_lo)

                idxt = metap.tile([P, T * 8], i16, tag="idx")
                mf = metap.tile([P, 2 * T], fp, tag="mf")
                nc.sync.dma_start(out=idxt[:], in_=idx_d[nt])
                nc.sync.dma_start(out=mf[:], in_=mf_d[nt])

                xg = xgp.tile([P, T * D], bf, tag="xg")
                gathers = []
                GMAX = 8  # 1024 idxs/call: larger overflows SWDGE desc ring
                qrr = nt  # rotate queue usage across node tiles
                for t0 in range(0, T_LO, GMAX):
                    tn = min(GMAX, T_LO - t0)
                    gathers.append(nc.gpsimd.dma_gather(
                        out_ap=xg[:, t0 * D:(t0 + tn) * D].rearrange(
                            "p (t f) -> p t f", f=D),
                        in_ap=xlo_d[:],
                        idxs_ap=idxt[:, t0 * 8:(t0 + tn) * 8],
                        num_idxs=P * tn,
                        num_idxs_reg=P * tn,
                        elem_size=D,
                        queue_num=qrr % 4,
                    ))
                    qrr += 1
                for t0 in range(T_LO, T, GMAX):
                    tn = min(GMAX, T - t0)
                    gathers.append(nc.gpsimd.dma_gather(
                        out_ap=xg[:, t0 * D:(t0 + tn) * D].rearrange(
                            "p (t f) -> p t f", f=D),
                        in_ap=xhi_d[:],
                        idxs_ap=idxt[:, t0 * 8:(t0 + tn) * 8],
                        num_idxs=P * tn,
                        num_idxs_reg=P * tn,
                        elem_size=D,
                        queue_num=qrr % 4,
                    ))
                    qrr += 1
                for g in gathers:
                    add_dep_helper(g.ins, fence.ins, reason="gather after x cast")

                psh = [pshp.tile([P, GRP * M], fp, tag=f"psh{g}", name=f"psh{g}")
                       for g in range(NGRP)]
                pso = psop.tile([P, D], fp)
                hsb = {}
                for k, (t, r, is_first, is_last) in enumerate(sched):
                    g = r // GRP
                    rloc = r - g * GRP
                    oh = selp.tile([P, M], bf, tag="oh")
                    nc.vector.tensor_scalar(
                        oh[:],
                        iota_sb[:],
                        mf[:, 2 * t:2 * t + 1],
                        mf[:, 2 * t + 1:2 * t + 2],
                        mybir.AluOpType.is_equal,
                        mybir.AluOpType.mult,
                    )
                    nc.tensor.matmul(
                        out=psh[g][:, rloc * M:(rloc + 1) * M],
                        lhsT=xg[:, t * D:(t + 1) * D],
                        rhs=oh[:],
                        start=is_first,
                        stop=is_last,
                    )
                    if k in grp_done:
                        gd = grp_done[k]
                        ht = hsbp.tile([P, GRP * M], bf, tag=f"hsb{gd}")
                        nc.scalar.activation(
                            out=ht[:], in_=psh[gd][:],
                            func=mybir.ActivationFunctionType.Copy,
                        )
                        hsb[gd] = ht

                for i, r in enumerate(active_rels):
                    g = r // GRP
                    rloc = r - g * GRP
                    nc.tensor.matmul(
                        out=pso[:m_sz, :],
                        lhsT=hsb[g][:, rloc * M:rloc * M + m_sz],
                        rhs=wr_sb[:, r * D:(r + 1) * D],
                        start=(i == 0),
                        stop=(i == len(active_rels) - 1),
                    )
                osb = osbp.tile([P, D], fp)
                nc.scalar.activation(
                    out=osb[:m_sz, :], in_=pso[:m_sz, :],
                    func=mybir.ActivationFunctionType.Copy,
                )
                nc.sync.dma_start(out=out_d[m_lo:m_lo + m_sz, :], in_=osb[:m_sz, :])
    nc.compile()
    return nc


def _wrap16(a):
    """Pack flat index array (n,) into dma_gather layout (128, n/16):
    index j lives at [j % 16, j // 16]; rows replicated to 128."""
    n = a.shape[0]
    w = a.reshape(n // 16, 16).T  # (16, n/16)
    return np.tile(w, (8, 1))


def kernel(x, source, target, edge_type, edge_weights, base_weights, bases):
    global LAST_PROFILE
    import ml_dtypes

    x = np.ascontiguousarray(np.asarray(x), dtype=np.float32)
    src = np.asarray(source).astype(np.int64)
    tgt = np.asarray(target).astype(np.int64)
    et = np.asarray(edge_type).astype(np.int64)
    ew = np.ascontiguousarray(np.asarray(edge_weights), dtype=np.float32)
    bw = np.ascontiguousarray(np.asarray(base_weights), dtype=np.float32)
    bs = np.ascontiguousarray(np.asarray(bases), dtype=np.float32)

    N, D = x.shape
    R, B = bw.shape
    E = src.shape[0]
    NPC = N // NCORES
    NT = (NPC + M - 1) // M

    # fold basis decomposition: W_r = sum_b bw[r,b] * bases[b]
    wr = np.einsum("rb,bio->rio", bw, bs).astype(ml_dtypes.bfloat16)

    # ---- host-side sharding: sort by (node-tile, src-half, relation) ----
    hi = (src >= SPLIT).astype(np.int64)
    core = tgt // NPC
    local = tgt - core * NPC
    ntile = local // M
    ntg = core * NT + ntile
    # src as final key: gather reads walk HBM monotonically within each run
    order = np.lexsort((src, et, hi, ntg))
    src_s = src[order]
    et_s = et[order]
    ew_s = ew[order]
    hi_s = hi[order]
    ntg_s = ntg[order]
    tgtf_s = (local[order] - (ntg_s % NT) * M).astype(np.float32)

    # per (core, nt, half, rel) counts -> uniform caps (max over core, nt)
    cell = (ntg_s * 2 + hi_s) * R + et_s  # monotone under the sort
    counts = np.bincount(cell, minlength=NCORES * NT * 2 * R)
    cnt4 = counts.reshape(NCORES * NT, 2, R)
    caps_lo = [int(np.ceil(cnt4[:, 0, r].max() / P)) for r in range(R)]
    caps_hi = [int(np.ceil(cnt4[:, 1, r].max() / P)) for r in range(R)]
    T_LO = sum(caps_lo)
    T_HI = sum(caps_hi)
    T = T_LO + T_HI

    # slot base of each (half, rel) run within a node tile
    run_base = np.zeros((2, R), dtype=np.int64)
    off = 0
    for r in range(R):
        run_base[0, r] = off
        off += caps_lo[r] * P
    for r in range(R):
        run_base[1, r] = off
        off += caps_hi[r] * P
    slots_per_nt = off  # == T * P

    starts = np.zeros(NCORES * NT * 2 * R + 1, dtype=np.int64)
    np.cumsum(counts, out=starts[1:])
    pos = np.arange(E, dtype=np.int64) - starts[cell]
    slot = ntg_s * slots_per_nt + run_base[hi_s, et_s] + pos

    nslots = NCORES * NT * slots_per_nt
    idx_flat = np.zeros(nslots, dtype=np.int16)
    tg_flat = np.zeros(nslots, dtype=np.float32)
    ew_flat = np.zeros(nslots, dtype=np.float32)
    idx_flat[slot] = (src_s - hi_s * SPLIT).astype(np.int16)
    tg_flat[slot] = tgtf_s
    ew_flat[slot] = ew_s

    # pad slots repeat the last real index of their (nt, half) section so the
    # gather's HBM reads stay monotone / page-local (ew=0 kills their output)
    real_pos = np.full(nslots, -1, dtype=np.int64)
    real_pos[slot] = slot
    np.maximum.accumulate(real_pos, out=real_pos)
    inslot = np.arange(nslots, dtype=np.int64) % slots_per_nt
    sec_start = (np.arange(nslots, dtype=np.int64) - inslot) + np.where(
        inslot < T_LO * P, 0, T_LO * P
    )
    fill_ok = real_pos >= sec_start
    idx_flat = np.where(fill_ok, idx_flat[np.maximum(real_pos, 0)], idx_flat)

    # dma_gather wrapped index layout per node tile: lo call then hi call
    idx16 = np.empty((NCORES, NT, P, T * 8), dtype=np.int16)
    idx_nt = idx_flat.reshape(NCORES, NT, slots_per_nt)
    for c in range(NCORES):
        for nt in range(NT):
            if T_LO:
                idx16[c, nt, :, :T_LO * 8] = _wrap16(idx_nt[c, nt, :T_LO * P])
            if T_HI:
                idx16[c, nt, :, T_LO * 8:] = _wrap16(idx_nt[c, nt, T_LO * P:])

    # meta_f: (C, NT, P, 4T) with [p, 4t..4t+3] = (tgt, -tgt, -ew, ew)
    tgr = tg_flat.reshape(NCORES, NT, T, P)
    ewr = ew_flat.reshape(NCORES, NT, T, P)
    mf5 = np.stack([tgr, ewr], axis=-1)  # (C, NT, T, P, 2)
    meta_f = np.ascontiguousarray(mf5.transpose(0, 1, 3, 2, 4)).reshape(
        NCORES, NT, P, 2 * T
    )

    iota_arr = np.ascontiguousarray(
        np.broadcast_to(np.arange(M, dtype=ml_dtypes.bfloat16), (P, M))
    )

    key = (N, D, R, NPC, NT, tuple(caps_lo), tuple(caps_hi))
    if key not in _PROG_CACHE:
        _PROG_CACHE[key] = _build_program(N, D, R, NPC, NT, caps_lo, caps_hi)
    nc = _PROG_CACHE[key]

    in_maps = [
        dict(
            x=x,
            wr16=wr,
            iota=iota_arr,
            idx16=idx16[c],
            meta_f=meta_f[c],
        )
        for c in range(NCORES)
    ]
    res = run_bass_kernel_spmd(nc, in_maps, list(range(NCORES)), trace=TRACE)
    LAST_PROFILE = res
    out = np.concatenate([res.results[c]["out"] for c in range(NCORES)], axis=0)
    return out


# revision 28
# speedup vs baseline: 1.1120x; 1.0020x over previous
"""BasesDecomposition GNN message passing on 8 Trainium2 NeuronCores.

Math (reference):
    seg  = edge_type * N + target
    h    = segment_sum(x[source] * ew, seg)        # (R, N, D)
    out  = einsum('rb,bio,rni->no', bw, bases, h)  # (N, D)

Restructuring: fold the basis decomposition into per-relation weight
matrices on the host:  W_r = sum_b bw[r,b] * bases[b]  (R, D, D).
Then  out[n] = sum_r h_r[n] @ W_r  with  h_r = segment_sum over edges of
relation r.  Per-edge work no longer involves the basis dimension, so the
device inner loop is:

    per 128-edge tile (single relation r):
        oh[e,m]     = (iota[m] == tgt_e) * ew_e        (1 DVE op, 128 wide)
        psum_h_r   += xg_tile^T @ oh                   (PE, accumulate)
    per 4-relation group: hsb = copy(psum_h)           (1 DVE op)
    per relation:  psum_out += hsb_r^T @ W_r           (PE, accumulate)

Sharding: nodes by target-id range across the 8 cores (no collective);
each core consumes only edges targeting its node range.  Edges sorted by
(node-tile, src-half, relation); each (tile, half, relation) run is padded
to 128-edge tiles with uniform caps (max over cores/tiles) so one SPMD
program serves all cores.  Gathers of x rows (bf16, split in lo/hi tables
for int16 indices) run on all 4 SWDGE queues (Q7 core pairs) round-robin.
"""

import numpy as np

import concourse.bass as bass
import concourse.mybir as mybir
import concourse.tile as tile
from concourse import bacc
from concourse.bass_utils import run_bass_kernel_spmd
from concourse.tile import add_dep_helper

NCORES = 8
P = 128          # edges per tile (matmul contraction dim)
M = 128          # nodes per node-tile (selector block width)
SPLIT = 32768    # x row split so gather indices fit int16
CAST_CHUNK = 1024  # x rows cast per prologue step
GRP = 4          # relations per PSUM bank group

TRACE = False
LAST_PROFILE = None

_PROG_CACHE = {}


def _build_program(N, D, R, NPC, NT, caps_lo, caps_hi):
    fp = mybir.dt.float32
    bf = mybir.dt.bfloat16
    i16 = mybir.dt.int16
    NHI = N - SPLIT

    T_LO = int(sum(caps_lo))
    T_HI = int(sum(caps_hi))
    T = T_LO + T_HI
    NGRP = (R + GRP - 1) // GRP

    # Layout: lo tiles rel-major (tile index = position in xg / mf), then hi
    # tiles rel-major.  Processing order is rel-consecutive (all tiles of rel
    # r back-to-back) so each PSUM region's accumulation group is sequential.
    lo_tiles = {}
    off = 0
    for r in range(R):
        lo_tiles[r] = list(range(off, off + caps_lo[r]))
        off += caps_lo[r]
    hi_tiles = {}
    for r in range(R):
        hi_tiles[r] = list(range(off, off + caps_hi[r]))
        off += caps_hi[r]
    active_rels = [r for r in range(R) if caps_lo[r] + caps_hi[r] > 0]
    sched = []  # (tile_index, rel, is_first_of_rel, is_last_of_rel)
    for r in active_rels:
        tiles_r = lo_tiles[r] + hi_tiles[r]
        for j, t in enumerate(tiles_r):
            sched.append((t, r, j == 0, j == len(tiles_r) - 1))
    # group g is complete after this position in sched
    grp_done = {}
    for g in range(NGRP):
        rels = [r for r in active_rels if g * GRP <= r < (g + 1) * GRP]
        if rels:
            last_r = max(rels)
            for k, (t, r, fi, la) in enumerate(sched):
                if r == last_r and la:
                    grp_done[k] = g

    nc = bacc.Bacc(
        "TRN2",
        target_bir_lowering=False,
        debug=False,
        num_devices=NCORES,
        num_swdge_queues=4,
    )
    x_d = nc.dram_tensor("x", [N, D], fp, kind="ExternalInput").ap()
    wr_d = nc.dram_tensor("wr16", [R, D, D], bf, kind="ExternalInput").ap()
    iota_d = nc.dram_tensor("iota", [P, M], bf, kind="ExternalInput").ap()
    idx_d = nc.dram_tensor("idx16", [NT, P, T * 8], i16, kind="ExternalInput").ap()
    # meta per tile t: cols 2t, 2t+1 = (tgt, ew)
    mf_d = nc.dram_tensor("meta_f", [NT, P, 2 * T], fp, kind="ExternalInput").ap()
    out_d = nc.dram_tensor("out", [NPC, D], fp, kind="ExternalOutput").ap()

    xlo_d = nc.dram_tensor("xlo", [SPLIT, D], bf).ap()
    xhi_d = nc.dram_tensor("xhi", [NHI, D], bf).ap()

    with tile.TileContext(nc) as tc:
        with (
            tc.tile_pool(name="const", bufs=1) as constp,
            tc.tile_pool(name="meta", bufs=3) as metap,
            tc.tile_pool(name="xg", bufs=2) as xgp,
            tc.tile_pool(name="sel", bufs=56) as selp,
            tc.tile_pool(name="hsb", bufs=2) as hsbp,
            tc.tile_pool(name="osb", bufs=3) as osbp,
            tc.tile_pool(name="psh", bufs=1, space="PSUM") as pshp,
            tc.tile_pool(name="pso", bufs=2, space="PSUM") as psop,
        ):
            iota_sb = constp.tile([P, M], bf)
            nc.sync.dma_start(out=iota_sb[:], in_=iota_d[:])
            wr_sb = constp.tile([P, R * D], bf)
            for r in range(R):
                nc.sync.dma_start(out=wr_sb[:, r * D:(r + 1) * D], in_=wr_d[r])

            # ---- prologue: bulk SWDGE cast x (f32) -> xlo/xhi (bf16), DRAM
            # to DRAM with dtype convert; no SBUF bounce ----
            cast_dmas = [
                nc.gpsimd.dma_start(out=xlo_d[:], in_=x_d[:SPLIT, :]),
                nc.gpsimd.dma_start(out=xhi_d[:], in_=x_d[SPLIT:, :]),
            ]
            fencet = constp.tile([P, 1], fp)
            fence = nc.gpsimd.memset(fencet[:], 0.0)
            for d in cast_dmas:
                add_dep_helper(fence.ins, d.ins, reason="x-cast fence")

            for nt in range(NT):
                m_lo = nt * M
                m_sz = min(M, NPC - m_lo)

                idxt = metap.tile([P, T * 8], i16, tag="idx")
                mf = metap.tile([P, 2 * T], fp, tag="mf")
                nc.sync.dma_start(out=idxt[:], in_=idx_d[nt])
                nc.sync.dma_start(out=mf[:], in_=mf_d[nt])

                xg = xgp.tile([P, T * D], bf, tag="xg")
                gathers = []
                GMAX = 8  # 1024 idxs/call: larger overflows SWDGE desc ring
                qrr = nt  # rotate queue usage across node tiles
                for t0 in range(0, T_LO, GMAX):
                    tn = min(GMAX, T_LO - t0)
                    gathers.append(nc.gpsimd.dma_gather(
                        out_ap=xg[:, t0 * D:(t0 + tn) * D].rearrange(
                            "p (t f) -> p t f", f=D),
                        in_ap=xlo_d[:],
                        idxs_ap=idxt[:, t0 * 8:(t0 + tn) * 8],
                        num_idxs=P * tn,
                        num_idxs_reg=P * tn,
                        elem_size=D,
                        queue_num=qrr % 4,
                    ))
                    qrr += 1
                for t0 in range(T_LO, T, GMAX):
                    tn = min(GMAX, T - t0)
                    gathers.append(nc.gpsimd.dma_gather(
                        out_ap=xg[:, t0 * D:(t0 + tn) * D].rearrange(
                            "p (t f) -> p t f", f=D),
                        in_ap=xhi_d[:],
                        idxs_ap=idxt[:, t0 * 8:(t0 + tn) * 8],
                        num_idxs=P * tn,
                        num_idxs_reg=P * tn,
                        elem_size=D,
                        queue_num=qrr % 4,
                    ))
                    qrr += 1
                for g in gathers:
                    add_dep_helper(g.ins, fence.ins, reason="gather after x cast")

                psh = [pshp.tile([P, GRP * M], fp, tag=f"psh{g}", name=f"psh{g}")
                       for g in range(NGRP)]
                pso = psop.tile([P, D], fp)
                hsb = {}
                for k, (t, r, is_first, is_last) in enumerate(sched):
                    g = r // GRP
                    rloc = r - g * GRP
                    oh = selp.tile([P, M], bf, tag="oh")
                    nc.vector.tensor_scalar(
                        oh[:],
                        iota_sb[:],
                        mf[:, 2 * t:2 * t + 1],
                        mf[:, 2 * t + 1:2 * t + 2],
                        mybir.AluOpType.is_equal,
                        mybir.AluOpType.mult,
                    )
                    nc.tensor.matmul(
                        out=psh[g][:, rloc * M:(rloc + 1) * M],
                        lhsT=xg[:, t * D:(t + 1) * D],
                        rhs=oh[:],
                        start=is_first,
                        stop=is_last,
                    )
                    if k in grp_done:
                        gd = grp_done[k]
                        ht = hsbp.tile([P, GRP * M], bf, tag=f"hsb{gd}")
                        nc.scalar.activation(
                            out=ht[:], in_=psh[gd][:],
                            func=mybir.ActivationFunctionType.Copy,
                        )
                        hsb[gd] = ht

                for i, r in enumerate(active_rels):
                    g = r // GRP
                    rloc = r - g * GRP
                    nc.tensor.matmul(
                        out=pso[:m_sz, :],
                        lhsT=hsb[g][:, rloc * M:rloc * M + m_sz],
                        rhs=wr_sb[:, r * D:(r + 1) * D],
                        start=(i == 0),
                        stop=(i == len(active_rels) - 1),
                    )
                osb = osbp.tile([P, D], fp)
                nc.scalar.activation(
                    out=osb[:m_sz, :], in_=pso[:m_sz, :],
                    func=mybir.ActivationFunctionType.Copy,
                )
                nc.sync.dma_start(out=out_d[m_lo:m_lo + m_sz, :], in_=osb[:m_sz, :])
    nc.compile()
    return nc


def _wrap16(a):
    """Pack flat index array (n,) into dma_gather layout (128, n/16):
    index j lives at [j % 16, j // 16]; rows replicated to 128."""
    n = a.shape[0]
    w = a.reshape(n // 16, 16).T  # (16, n/16)
    return np.tile(w, (8, 1))


def kernel(x, source, target, edge_type, edge_weights, base_weights, bases):
    global LAST_PROFILE
    import ml_dtypes

    x = np.ascontiguousarray(np.asarray(x), dtype=np.float32)
    src = np.asarray(source).astype(np.int64)
    tgt = np.asarray(target).astype(np.int64)
    et = np.asarray(edge_type).astype(np.int64)
    ew = np.ascontiguousarray(np.asarray(edge_weights), dtype=np.float32)
    bw = np.ascontiguousarray(np.asarray(base_weights), dtype=np.float32)
    bs = np.ascontiguousarray(np.asarray(bases), dtype=np.float32)

    N, D = x.shape
    R, B = bw.shape
    E = src.shape[0]
    NPC = N // NCORES
    NT = (NPC + M - 1) // M

    # fold basis decomposition: W_r = sum_b bw[r,b] * bases[b]
    wr = np.einsum("rb,bio->rio", bw, bs).astype(ml_dtypes.bfloat16)

    # ---- host-side sharding: sort by (node-tile, src-half, relation) ----
    hi = (src >= SPLIT).astype(np.int64)
    core = tgt // NPC
    local = tgt - core * NPC
    ntile = local // M
    ntg = core * NT + ntile
    # src as final key: gather reads walk HBM monotonically within each run
    order = np.lexsort((src, et, hi, ntg))
    src_s = src[order]
    et_s = et[order]
    ew_s = ew[order]
    hi_s = hi[order]
    ntg_s = ntg[order]
    tgtf_s = (local[order] - (ntg_s % NT) * M).astype(np.float32)

    # per (core, nt, half, rel) counts -> uniform caps (max over core, nt)
    cell = (ntg_s * 2 + hi_s) * R + et_s  # monotone under the sort
    counts = np.bincount(cell, minlength=NCORES * NT * 2 * R)
    cnt4 = counts.reshape(NCORES * NT, 2, R)
    caps_lo = [int(np.ceil(cnt4[:, 0, r].max() / P)) for r in range(R)]
    caps_hi = [int(np.ceil(cnt4[:, 1, r].max() / P)) for r in range(R)]
    T_LO = sum(caps_lo)
    T_HI = sum(caps_hi)
    T = T_LO + T_HI

    # slot base of each (half, rel) run within a node tile
    run_base = np.zeros((2, R), dtype=np.int64)
    off = 0
    for r in range(R):
        run_base[0, r] = off
        off += caps_lo[r] * P
    for r in range(R):
        run_base[1, r] = off
        off += caps_hi[r] * P
    slots_per_nt = off  # == T * P

    starts = np.zeros(NCORES * NT * 2 * R + 1, dtype=np.int64)
    np.cumsum(counts, out=starts[1:])
    pos = np.arange(E, dtype=np.int64) - starts[cell]
    slot = ntg_s * slots_per_nt + run_base[hi_s, et_s] + pos

    nslots = NCORES * NT * slots_per_nt
    idx_flat = np.zeros(nslots, dtype=np.int16)
    tg_flat = np.zeros(nslots, dtype=np.float32)
    ew_flat = np.zeros(nslots, dtype=np.float32)
    idx_flat[slot] = (src_s - hi_s * SPLIT).astype(np.int16)
    tg_flat[slot] = tgtf_s
    ew_flat[slot] = ew_s

    # pad slots repeat the last real index of their (nt, half) section so the
    # gather's HBM reads stay monotone / page-local (ew=0 kills their output)
    real_pos = np.full(nslots, -1, dtype=np.int64)
    real_pos[slot] = slot
    np.maximum.accumulate(real_pos, out=real_pos)
    inslot = np.arange(nslots, dtype=np.int64) % slots_per_nt
    sec_start = (np.arange(nslots, dtype=np.int64) - inslot) + np.where(
        inslot < T_LO * P, 0, T_LO * P
    )
    fill_ok = real_pos >= sec_start
    idx_flat = np.where(fill_ok, idx_flat[np.maximum(real_pos, 0)], idx_flat)

    # dma_gather wrapped index layout per node tile: lo call then hi call
    idx16 = np.empty((NCORES, NT, P, T * 8), dtype=np.int16)
    idx_nt = idx_flat.reshape(NCORES, NT, slots_per_nt)
    for c in range(NCORES):
        for nt in range(NT):
            if T_LO:
                idx16[c, nt, :, :T_LO * 8] = _wrap16(idx_nt[c, nt, :T_LO * P])
            if T_HI:
                idx16[c, nt, :, T_LO * 8:] = _wrap16(idx_nt[c, nt, T_LO * P:])

    # meta_f: (C, NT, P, 4T) with [p, 4t..4t+3] = (tgt, -tgt, -ew, ew)
    tgr = tg_flat.reshape(NCORES, NT, T, P)
    ewr = ew_flat.reshape(NCORES, NT, T, P)
    mf5 = np.stack([tgr, ewr], axis=-1)  # (C, NT, T, P, 2)
    meta_f = np.ascontiguousarray(mf5.transpose(0, 1, 3, 2, 4)).reshape(
        NCORES, NT, P, 2 * T
    )

    iota_arr = np.ascontiguousarray(
        np.broadcast_to(np.arange(M, dtype=ml_dtypes.bfloat16), (P, M))
    )

    key = (N, D, R, NPC, NT, tuple(caps_lo), tuple(caps_hi))
    if key not in _PROG_CACHE:
        _PROG_CACHE[key] = _build_program(N, D, R, NPC, NT, caps_lo, caps_hi)
    nc = _PROG_CACHE[key]

    in_maps = [
        dict(
            x=x,
            wr16=wr,
            iota=iota_arr,
            idx16=idx16[c],
            meta_f=meta_f[c],
        )
        for c in range(NCORES)
    ]
    res = run_bass_kernel_spmd(nc, in_maps, list(range(NCORES)), trace=TRACE)
    LAST_PROFILE = res
    out = np.concatenate([res.results[c]["out"] for c in range(NCORES)], axis=0)
    return out


# revision 30
# speedup vs baseline: 1.1230x; 1.0099x over previous
"""BasesDecomposition GNN message passing on 8 Trainium2 NeuronCores.

Math (reference):
    seg  = edge_type * N + target
    h    = segment_sum(x[source] * ew, seg)        # (R, N, D)
    out  = einsum('rb,bio,rni->no', bw, bases, h)  # (N, D)

Restructuring: fold the basis decomposition into per-relation weight
matrices on the host:  W_r = sum_b bw[r,b] * bases[b]  (R, D, D).
Then  out[n] = sum_r h_r[n] @ W_r  with  h_r = segment_sum over edges of
relation r.  Per-edge work no longer involves the basis dimension, so the
device inner loop is:

    per 128-edge tile (single relation r):
        oh[e,m]     = (iota[m] == tgt_e) * ew_e        (1 DVE op, 128 wide)
        psum_h_r   += xg_tile^T @ oh                   (PE, accumulate)
    per 4-relation group: hsb = copy(psum_h)           (1 DVE op)
    per relation:  psum_out += hsb_r^T @ W_r           (PE, accumulate)

Sharding: nodes by target-id range across the 8 cores (no collective);
each core consumes only edges targeting its node range.  Edges sorted by
(node-tile, src-half, relation); each (tile, half, relation) run is padded
to 128-edge tiles with uniform caps (max over cores/tiles) so one SPMD
program serves all cores.  Gathers of x rows (bf16, split in lo/hi tables
for int16 indices) run on all 4 SWDGE queues (Q7 core pairs) round-robin.
"""

import numpy as np

import concourse.bass as bass
import concourse.mybir as mybir
import concourse.tile as tile
from concourse import bacc
from concourse.bass_utils import run_bass_kernel_spmd
from concourse.tile import add_dep_helper

NCORES = 8
P = 128          # edges per tile (matmul contraction dim)
M = 128          # nodes per node-tile (selector block width)
SPLIT = 32768    # x row split so gather indices fit int16
CAST_CHUNK = 1024  # x rows cast per prologue step
GRP = 4          # relations per PSUM bank group

TRACE = False
LAST_PROFILE = None

_PROG_CACHE = {}


def _build_program(N, D, R, NPC, NT, caps_lo, caps_hi):
    fp = mybir.dt.float32
    bf = mybir.dt.bfloat16
    i16 = mybir.dt.int16
    NHI = N - SPLIT

    T_LO = int(sum(caps_lo))
    T_HI = int(sum(caps_hi))
    T = T_LO + T_HI
    NGRP = (R + GRP - 1) // GRP

    # Layout: lo tiles rel-major (tile index = position in xg / mf), then hi
    # tiles rel-major.  Processing order is rel-consecutive (all tiles of rel
    # r back-to-back) so each PSUM region's accumulation group is sequential.
    lo_tiles = {}
    off = 0
    for r in range(R):
        lo_tiles[r] = list(range(off, off + caps_lo[r]))
        off += caps_lo[r]
    hi_tiles = {}
    for r in range(R):
        hi_tiles[r] = list(range(off, off + caps_hi[r]))
        off += caps_hi[r]
    active_rels = [r for r in range(R) if caps_lo[r] + caps_hi[r] > 0]
    sched = []  # (tile_index, rel, is_first_of_rel, is_last_of_rel)
    for r in active_rels:
        tiles_r = lo_tiles[r] + hi_tiles[r]
        for j, t in enumerate(tiles_r):
            sched.append((t, r, j == 0, j == len(tiles_r) - 1))
    # group g is complete after this position in sched
    grp_done = {}
    for g in range(NGRP):
        rels = [r for r in active_rels if g * GRP <= r < (g + 1) * GRP]
        if rels:
            last_r = max(rels)
            for k, (t, r, fi, la) in enumerate(sched):
                if r == last_r and la:
                    grp_done[k] = g

    nc = bacc.Bacc(
        "TRN2",
        target_bir_lowering=False,
        debug=False,
        num_devices=NCORES,
        num_swdge_queues=4,
    )
    x_d = nc.dram_tensor("x", [N, D], fp, kind="ExternalInput").ap()
    wr_d = nc.dram_tensor("wr16", [R, D, D], bf, kind="ExternalInput").ap()
    iota_d = nc.dram_tensor("iota", [P, M], bf, kind="ExternalInput").ap()
    idx_d = nc.dram_tensor("idx16", [NT, P, T * 8], i16, kind="ExternalInput").ap()
    # meta per tile t: cols 2t, 2t+1 = (tgt, ew)
    mf_d = nc.dram_tensor("meta_f", [NT, P, 2 * T], fp, kind="ExternalInput").ap()
    out_d = nc.dram_tensor("out", [NPC, D], fp, kind="ExternalOutput").ap()

    xlo_d = nc.dram_tensor("xlo", [SPLIT, D], bf).ap()
    xhi_d = nc.dram_tensor("xhi", [NHI, D], bf).ap()

    with tile.TileContext(nc) as tc:
        with (
            tc.tile_pool(name="const", bufs=1) as constp,
            tc.tile_pool(name="meta", bufs=6) as metap,
            tc.tile_pool(name="xg", bufs=3) as xgp,
            tc.tile_pool(name="sel", bufs=56) as selp,
            tc.tile_pool(name="hsb", bufs=2) as hsbp,
            tc.tile_pool(name="osb", bufs=3) as osbp,
            tc.tile_pool(name="psh", bufs=1, space="PSUM") as pshp,
            tc.tile_pool(name="pso", bufs=2, space="PSUM") as psop,
        ):
            iota_sb = constp.tile([P, M], bf)
            nc.sync.dma_start(out=iota_sb[:], in_=iota_d[:])
            wr_sb = constp.tile([P, R * D], bf)
            for r in range(R):
                nc.sync.dma_start(out=wr_sb[:, r * D:(r + 1) * D], in_=wr_d[r])

            # ---- prologue: bulk SWDGE cast x (f32) -> xlo/xhi (bf16), DRAM
            # to DRAM with dtype convert; no SBUF bounce.  Separate fences so
            # lo-gathers only wait for the lo cast (and vice versa). ----
            cast_lo = nc.gpsimd.dma_start(out=xlo_d[:], in_=x_d[:SPLIT, :])
            cast_hi = nc.gpsimd.dma_start(out=xhi_d[:], in_=x_d[SPLIT:, :])
            fencet = constp.tile([P, 2], fp)
            fence_lo = nc.gpsimd.memset(fencet[:, 0:1], 0.0)
            fence_hi = nc.gpsimd.memset(fencet[:, 1:2], 0.0)
            add_dep_helper(fence_lo.ins, cast_lo.ins, reason="xlo-cast fence")
            add_dep_helper(fence_hi.ins, cast_hi.ins, reason="xhi-cast fence")

            for nt in range(NT):
                m_lo = nt * M
                m_sz = min(M, NPC - m_lo)

                idxt = metap.tile([P, T * 8], i16, tag="idx")
                mf = metap.tile([P, 2 * T], fp, tag="mf")
                nc.sync.dma_start(out=idxt[:], in_=idx_d[nt])
                nc.sync.dma_start(out=mf[:], in_=mf_d[nt])

                xg = xgp.tile([P, T * D], bf, tag="xg")
                gathers = []
                GMAX = 8  # 1024 idxs/call: larger overflows SWDGE desc ring
                n_lo_calls = (T_LO + GMAX - 1) // GMAX
                qrr = nt  # rotate queue usage across node tiles
                for t0 in range(0, T_LO, GMAX):
                    tn = min(GMAX, T_LO - t0)
                    gathers.append(nc.gpsimd.dma_gather(
                        out_ap=xg[:, t0 * D:(t0 + tn) * D].rearrange(
                            "p (t f) -> p t f", f=D),
                        in_ap=xlo_d[:],
                        idxs_ap=idxt[:, t0 * 8:(t0 + tn) * 8],
                        num_idxs=P * tn,
                        num_idxs_reg=P * tn,
                        elem_size=D,
                        queue_num=qrr % 4,
                    ))
                    qrr += 1
                for t0 in range(T_LO, T, GMAX):
                    tn = min(GMAX, T - t0)
                    gathers.append(nc.gpsimd.dma_gather(
                        out_ap=xg[:, t0 * D:(t0 + tn) * D].rearrange(
                            "p (t f) -> p t f", f=D),
                        in_ap=xhi_d[:],
                        idxs_ap=idxt[:, t0 * 8:(t0 + tn) * 8],
                        num_idxs=P * tn,
                        num_idxs_reg=P * tn,
                        elem_size=D,
                        queue_num=qrr % 4,
                    ))
                    qrr += 1
                for i, g in enumerate(gathers):
                    f = fence_lo if i < n_lo_calls else fence_hi
                    add_dep_helper(g.ins, f.ins, reason="gather after x cast")

                psh = [pshp.tile([P, GRP * M], fp, tag=f"psh{g}", name=f"psh{g}")
                       for g in range(NGRP)]
                pso = psop.tile([P, D], fp)
                hsb = {}
                for k, (t, r, is_first, is_last) in enumerate(sched):
                    g = r // GRP
                    rloc = r - g * GRP
                    oh = selp.tile([P, M], bf, tag="oh")
                    nc.vector.tensor_scalar(
                        oh[:],
                        iota_sb[:],
                        mf[:, 2 * t:2 * t + 1],
                        mf[:, 2 * t + 1:2 * t + 2],
                        mybir.AluOpType.is_equal,
                        mybir.AluOpType.mult,
                    )
                    nc.tensor.matmul(
                        out=psh[g][:, rloc * M:(rloc + 1) * M],
                        lhsT=xg[:, t * D:(t + 1) * D],
                        rhs=oh[:],
                        start=is_first,
                        stop=is_last,
                    )
                    if k in grp_done:
                        gd = grp_done[k]
                        ht = hsbp.tile([P, GRP * M], bf, tag=f"hsb{gd}")
                        nc.scalar.activation(
                            out=ht[:], in_=psh[gd][:],
                            func=mybir.ActivationFunctionType.Copy,
                        )
                        hsb[gd] = ht

                for i, r in enumerate(active_rels):
                    g = r // GRP
                    rloc = r - g * GRP
                    nc.tensor.matmul(
                        out=pso[:m_sz, :],
                        lhsT=hsb[g][:, rloc * M:rloc * M + m_sz],
                        rhs=wr_sb[:, r * D:(r + 1) * D],
                        start=(i == 0),
                        stop=(i == len(active_rels) - 1),
                    )
                osb = osbp.tile([P, D], fp)
                nc.scalar.activation(
                    out=osb[:m_sz, :], in_=pso[:m_sz, :],
                    func=mybir.ActivationFunctionType.Copy,
                )
                nc.sync.dma_start(out=out_d[m_lo:m_lo + m_sz, :], in_=osb[:m_sz, :])
    nc.compile()
    return nc


def _wrap16(a):
    """Pack flat index array (n,) into dma_gather layout (128, n/16):
    index j lives at [j % 16, j // 16]; rows replicated to 128."""
    n = a.shape[0]
    w = a.reshape(n // 16, 16).T  # (16, n/16)
    return np.tile(w, (8, 1))


def kernel(x, source, target, edge_type, edge_weights, base_weights, bases):
    global LAST_PROFILE
    import ml_dtypes

    x = np.ascontiguousarray(np.asarray(x), dtype=np.float32)
    src = np.asarray(source).astype(np.int64)
    tgt = np.asarray(target).astype(np.int64)
    et = np.asarray(edge_type).astype(np.int64)
    ew = np.ascontiguousarray(np.asarray(edge_weights), dtype=np.float32)
    bw = np.ascontiguousarray(np.asarray(base_weights), dtype=np.float32)
    bs = np.ascontiguousarray(np.asarray(bases), dtype=np.float32)

    N, D = x.shape
    R, B = bw.shape
    E = src.shape[0]
    NPC = N // NCORES
    NT = (NPC + M - 1) // M

    # fold basis decomposition: W_r = sum_b bw[r,b] * bases[b]
    wr = np.einsum("rb,bio->rio", bw, bs).astype(ml_dtypes.bfloat16)

    # ---- host-side sharding: sort by (node-tile, src-half, relation) ----
    hi = (src >= SPLIT).astype(np.int64)
    core = tgt // NPC
    local = tgt - core * NPC
    ntile = local // M
    ntg = core * NT + ntile
    # src as final key: gather reads walk HBM monotonically within each run
    order = np.lexsort((src, et, hi, ntg))
    src_s = src[order]
    et_s = et[order]
    ew_s = ew[order]
    hi_s = hi[order]
    ntg_s = ntg[order]
    tgtf_s = (local[order] - (ntg_s % NT) * M).astype(np.float32)

    # per (core, nt, half, rel) counts -> uniform caps (max over core, nt)
    cell = (ntg_s * 2 + hi_s) * R + et_s  # monotone under the sort
    counts = np.bincount(cell, minlength=NCORES * NT * 2 * R)
    cnt4 = counts.reshape(NCORES * NT, 2, R)
    caps_lo = [int(np.ceil(cnt4[:, 0, r].max() / P)) for r in range(R)]
    caps_hi = [int(np.ceil(cnt4[:, 1, r].max() / P)) for r in range(R)]
    T_LO = sum(caps_lo)
    T_HI = sum(caps_hi)
    T = T_LO + T_HI

    # slot base of each (half, rel) run within a node tile
    run_base = np.zeros((2, R), dtype=np.int64)
    off = 0
    for r in range(R):
        run_base[0, r] = off
        off += caps_lo[r] * P
    for r in range(R):
        run_base[1, r] = off
        off += caps_hi[r] * P
    slots_per_nt = off  # == T * P

    starts = np.zeros(NCORES * NT * 2 * R + 1, dtype=np.int64)
    np.cumsum(counts, out=starts[1:])
    pos = np.arange(E, dtype=np.int64) - starts[cell]
    slot = ntg_s * slots_per_nt + run_base[hi_s, et_s] + pos

    nslots = NCORES * NT * slots_per_nt
    idx_flat = np.zeros(nslots, dtype=np.int16)
    tg_flat = np.zeros(nslots, dtype=np.float32)
    ew_flat = np.zeros(nslots, dtype=np.float32)
    idx_flat[slot] = (src_s - hi_s * SPLIT).astype(np.int16)
    tg_flat[slot] = tgtf_s
    ew_flat[slot] = ew_s

    # pad slots repeat the last real index of their (nt, half) section so the
    # gather's HBM reads stay monotone / page-local (ew=0 kills their output)
    real_pos = np.full(nslots, -1, dtype=np.int64)
    real_pos[slot] = slot
    np.maximum.accumulate(real_pos, out=real_pos)
    inslot = np.arange(nslots, dtype=np.int64) % slots_per_nt
    sec_start = (np.arange(nslots, dtype=np.int64) - inslot) + np.where(
        inslot < T_LO * P, 0, T_LO * P
    )
    fill_ok = real_pos >= sec_start
    idx_flat = np.where(fill_ok, idx_flat[np.maximum(real_pos, 0)], idx_flat)

    # dma_gather wrapped index layout per node tile: lo call then hi call
    idx16 = np.empty((NCORES, NT, P, T * 8), dtype=np.int16)
    idx_nt = idx_flat.reshape(NCORES, NT, slots_per_nt)
    for c in range(NCORES):
        for nt in range(NT):
            if T_LO:
                idx16[c, nt, :, :T_LO * 8] = _wrap16(idx_nt[c, nt, :T_LO * P])
            if T_HI:
                idx16[c, nt, :, T_LO * 8:] = _wrap16(idx_nt[c, nt, T_LO * P:])

    # meta_f: (C, NT, P, 4T) with [p, 4t..4t+3] = (tgt, -tgt, -ew, ew)
    tgr = tg_flat.reshape(NCORES, NT, T, P)
    ewr = ew_flat.reshape(NCORES, NT, T, P)
    mf5 = np.stack([tgr, ewr], axis=-1)  # (C, NT, T, P, 2)
    meta_f = np.ascontiguousarray(mf5.transpose(0, 1, 3, 2, 4)).reshape(
        NCORES, NT, P, 2 * T
    )

    iota_arr = np.ascontiguousarray(
        np.broadcast_to(np.arange(M, dtype=ml_dtypes.bfloat16), (P, M))
    )

    key = (N, D, R, NPC, NT, tuple(caps_lo), tuple(caps_hi))
    if key not in _PROG_CACHE:
        _PROG_CACHE[key] = _build_program(N, D, R, NPC, NT, caps_lo, caps_hi)
    nc = _PROG_CACHE[key]

    in_maps = [
        dict(
            x=x,
            wr16=wr,
            iota=iota_arr,
            idx16=idx16[c],
            meta_f=meta_f[c],
        )
        for c in range(NCORES)
    ]
    res = run_bass_kernel_spmd(nc, in_maps, list(range(NCORES)), trace=TRACE)
    LAST_PROFILE = res
    out = np.concatenate([res.results[c]["out"] for c in range(NCORES)], axis=0)
    return out


# revision 31
# speedup vs baseline: 1.1368x; 1.0123x over previous
"""BasesDecomposition GNN message passing on 8 Trainium2 NeuronCores.

Math (reference):
    seg  = edge_type * N + target
    h    = segment_sum(x[source] * ew, seg)        # (R, N, D)
    out  = einsum('rb,bio,rni->no', bw, bases, h)  # (N, D)

Restructuring: fold the basis decomposition into per-relation weight
matrices on the host:  W_r = sum_b bw[r,b] * bases[b]  (R, D, D).
Then  out[n] = sum_r h_r[n] @ W_r  with  h_r = segment_sum over edges of
relation r.  Per-edge work no longer involves the basis dimension, so the
device inner loop is:

    per 128-edge tile (single relation r):
        oh[e,m]     = (iota[m] == tgt_e) * ew_e        (1 DVE op, 128 wide)
        psum_h_r   += xg_tile^T @ oh                   (PE, accumulate)
    per 4-relation group: hsb = copy(psum_h)           (1 DVE op)
    per relation:  psum_out += hsb_r^T @ W_r           (PE, accumulate)

Sharding: nodes by target-id range across the 8 cores (no collective);
each core consumes only edges targeting its node range.  Edges sorted by
(node-tile, src-half, relation); each (tile, half, relation) run is padded
to 128-edge tiles with uniform caps (max over cores/tiles) so one SPMD
program serves all cores.  Gathers of x rows (bf16, split in lo/hi tables
for int16 indices) run on all 4 SWDGE queues (Q7 core pairs) round-robin.
"""

import numpy as np

import concourse.bass as bass
import concourse.mybir as mybir
import concourse.tile as tile
from concourse import bacc
from concourse.bass_utils import run_bass_kernel_spmd
from concourse.tile import add_dep_helper

NCORES = 8
P = 128          # edges per tile (matmul contraction dim)
M = 128          # nodes per node-tile (selector block width)
SPLIT = 32768    # x row split so gather indices fit int16
CAST_CHUNK = 1024  # x rows cast per prologue step
GRP = 4          # relations per PSUM bank group

TRACE = False
LAST_PROFILE = None

_PROG_CACHE = {}


def _build_program(N, D, R, NPC, NT, caps_lo, caps_hi):
    fp = mybir.dt.float32
    bf = mybir.dt.bfloat16
    i16 = mybir.dt.int16
    NHI = N - SPLIT

    T_LO = int(sum(caps_lo))
    T_HI = int(sum(caps_hi))
    T = T_LO + T_HI
    NGRP = (R + GRP - 1) // GRP

    # Layout: lo tiles rel-major (tile index = position in xg / mf), then hi
    # tiles rel-major.  Processing order is rel-consecutive (all tiles of rel
    # r back-to-back) so each PSUM region's accumulation group is sequential.
    lo_tiles = {}
    off = 0
    for r in range(R):
        lo_tiles[r] = list(range(off, off + caps_lo[r]))
        off += caps_lo[r]
    hi_tiles = {}
    for r in range(R):
        hi_tiles[r] = list(range(off, off + caps_hi[r]))
        off += caps_hi[r]
    active_rels = [r for r in range(R) if caps_lo[r] + caps_hi[r] > 0]
    sched = []  # (tile_index, rel, is_first_of_rel, is_last_of_rel)
    for r in active_rels:
        tiles_r = lo_tiles[r] + hi_tiles[r]
        for j, t in enumerate(tiles_r):
            sched.append((t, r, j == 0, j == len(tiles_r) - 1))
    # group g is complete after this position in sched
    grp_done = {}
    for g in range(NGRP):
        rels = [r for r in active_rels if g * GRP <= r < (g + 1) * GRP]
        if rels:
            last_r = max(rels)
            for k, (t, r, fi, la) in enumerate(sched):
                if r == last_r and la:
                    grp_done[k] = g

    nc = bacc.Bacc(
        "TRN2",
        target_bir_lowering=False,
        debug=False,
        num_devices=NCORES,
        num_swdge_queues=4,
    )
    x_d = nc.dram_tensor("x", [N, D], fp, kind="ExternalInput").ap()
    wr_d = nc.dram_tensor("wr16", [R, D, D], bf, kind="ExternalInput").ap()
    iota_d = nc.dram_tensor("iota", [P, M], bf, kind="ExternalInput").ap()
    idx_d = nc.dram_tensor("idx16", [NT, P, T * 8], i16, kind="ExternalInput").ap()
    # meta per tile t: cols 2t, 2t+1 = (tgt, ew)
    mf_d = nc.dram_tensor("meta_f", [NT, P, 2 * T], fp, kind="ExternalInput").ap()
    out_d = nc.dram_tensor("out", [NPC, D], fp, kind="ExternalOutput").ap()

    xlo_d = nc.dram_tensor("xlo", [SPLIT, D], bf).ap()
    xhi_d = nc.dram_tensor("xhi", [NHI, D], bf).ap()

    with tile.TileContext(nc) as tc:
        with (
            tc.tile_pool(name="const", bufs=1) as constp,
            tc.tile_pool(name="meta", bufs=8) as metap,
            tc.tile_pool(name="xg", bufs=4) as xgp,
            tc.tile_pool(name="sel", bufs=100) as selp,
            tc.tile_pool(name="hsb", bufs=2) as hsbp,
            tc.tile_pool(name="osb", bufs=3) as osbp,
            tc.tile_pool(name="psh", bufs=1, space="PSUM") as pshp,
            tc.tile_pool(name="pso", bufs=2, space="PSUM") as psop,
        ):
            iota_sb = constp.tile([P, M], bf)
            nc.sync.dma_start(out=iota_sb[:], in_=iota_d[:])
            wr_sb = constp.tile([P, R * D], bf)
            for r in range(R):
                nc.sync.dma_start(out=wr_sb[:, r * D:(r + 1) * D], in_=wr_d[r])

            # ---- prologue: bulk SWDGE cast x (f32) -> xlo/xhi (bf16), DRAM
            # to DRAM with dtype convert; no SBUF bounce.  Separate fences so
            # lo-gathers only wait for the lo cast (and vice versa). ----
            cast_lo = nc.gpsimd.dma_start(out=xlo_d[:], in_=x_d[:SPLIT, :])
            cast_hi = nc.gpsimd.dma_start(out=xhi_d[:], in_=x_d[SPLIT:, :])
            fencet = constp.tile([P, 2], fp)
            fence_lo = nc.gpsimd.memset(fencet[:, 0:1], 0.0)
            fence_hi = nc.gpsimd.memset(fencet[:, 1:2], 0.0)
            add_dep_helper(fence_lo.ins, cast_lo.ins, reason="xlo-cast fence")
            add_dep_helper(fence_hi.ins, cast_hi.ins, reason="xhi-cast fence")

            for nt in range(NT):
                m_lo = nt * M
                m_sz = min(M, NPC - m_lo)

                idxt = metap.tile([P, T * 8], i16, tag="idx")
                mf = metap.tile([P, 2 * T], fp, tag="mf")
                nc.sync.dma_start(out=idxt[:], in_=idx_d[nt])
                nc.sync.dma_start(out=mf[:], in_=mf_d[nt])

                xg = xgp.tile([P, T * D], bf, tag="xg")
                gathers = []
                GMAX = 8  # 1024 idxs/call: larger overflows SWDGE desc ring
                n_lo_calls = (T_LO + GMAX - 1) // GMAX
                qrr = nt  # rotate queue usage across node tiles
                for t0 in range(0, T_LO, GMAX):
                    tn = min(GMAX, T_LO - t0)
                    gathers.append(nc.gpsimd.dma_gather(
                        out_ap=xg[:, t0 * D:(t0 + tn) * D].rearrange(
                            "p (t f) -> p t f", f=D),
                        in_ap=xlo_d[:],
                        idxs_ap=idxt[:, t0 * 8:(t0 + tn) * 8],
                        num_idxs=P * tn,
                        num_idxs_reg=P * tn,
                        elem_size=D,
                        queue_num=qrr % 4,
                    ))
                    qrr += 1
                for t0 in range(T_LO, T, GMAX):
                    tn = min(GMAX, T - t0)
                    gathers.append(nc.gpsimd.dma_gather(
                        out_ap=xg[:, t0 * D:(t0 + tn) * D].rearrange(
                            "p (t f) -> p t f", f=D),
                        in_ap=xhi_d[:],
                        idxs_ap=idxt[:, t0 * 8:(t0 + tn) * 8],
                        num_idxs=P * tn,
                        num_idxs_reg=P * tn,
                        elem_size=D,
                        queue_num=qrr % 4,
                    ))
                    qrr += 1
                for i, g in enumerate(gathers):
                    f = fence_lo if i < n_lo_calls else fence_hi
                    add_dep_helper(g.ins, f.ins, reason="gather after x cast")

                psh = [pshp.tile([P, GRP * M], fp, tag=f"psh{g}", name=f"psh{g}")
                       for g in range(NGRP)]
                pso = psop.tile([P, D], fp)
                hsb = {}
                for k, (t, r, is_first, is_last) in enumerate(sched):
                    g = r // GRP
                    rloc = r - g * GRP
                    oh = selp.tile([P, M], bf, tag="oh")
                    nc.vector.tensor_scalar(
                        oh[:],
                        iota_sb[:],
                        mf[:, 2 * t:2 * t + 1],
                        mf[:, 2 * t + 1:2 * t + 2],
                        mybir.AluOpType.is_equal,
                        mybir.AluOpType.mult,
                    )
                    nc.tensor.matmul(
                        out=psh[g][:, rloc * M:(rloc + 1) * M],
                        lhsT=xg[:, t * D:(t + 1) * D],
                        rhs=oh[:],
                        start=is_first,
                        stop=is_last,
                    )
                    if k in grp_done:
                        gd = grp_done[k]
                        ht = hsbp.tile([P, GRP * M], bf, tag=f"hsb{gd}")
                        nc.scalar.activation(
                            out=ht[:], in_=psh[gd][:],
                            func=mybir.ActivationFunctionType.Copy,
                        )
                        hsb[gd] = ht

                for i, r in enumerate(active_rels):
                    g = r // GRP
                    rloc = r - g * GRP
                    nc.tensor.matmul(
                        out=pso[:m_sz, :],
                        lhsT=hsb[g][:, rloc * M:rloc * M + m_sz],
                        rhs=wr_sb[:, r * D:(r + 1) * D],
                        start=(i == 0),
                        stop=(i == len(active_rels) - 1),
                    )
                osb = osbp.tile([P, D], fp)
                nc.scalar.activation(
                    out=osb[:m_sz, :], in_=pso[:m_sz, :],
                    func=mybir.ActivationFunctionType.Copy,
                )
                nc.sync.dma_start(out=out_d[m_lo:m_lo + m_sz, :], in_=osb[:m_sz, :])
    nc.compile()
    return nc


def _wrap16(a):
    """Pack flat index array (n,) into dma_gather layout (128, n/16):
    index j lives at [j % 16, j // 16]; rows replicated to 128."""
    n = a.shape[0]
    w = a.reshape(n // 16, 16).T  # (16, n/16)
    return np.tile(w, (8, 1))


def kernel(x, source, target, edge_type, edge_weights, base_weights, bases):
    global LAST_PROFILE
    import ml_dtypes

    x = np.ascontiguousarray(np.asarray(x), dtype=np.float32)
    src = np.asarray(source).astype(np.int64)
    tgt = np.asarray(target).astype(np.int64)
    et = np.asarray(edge_type).astype(np.int64)
    ew = np.ascontiguousarray(np.asarray(edge_weights), dtype=np.float32)
    bw = np.ascontiguousarray(np.asarray(base_weights), dtype=np.float32)
    bs = np.ascontiguousarray(np.asarray(bases), dtype=np.float32)

    N, D = x.shape
    R, B = bw.shape
    E = src.shape[0]
    NPC = N // NCORES
    NT = (NPC + M - 1) // M

    # fold basis decomposition: W_r = sum_b bw[r,b] * bases[b]
    wr = np.einsum("rb,bio->rio", bw, bs).astype(ml_dtypes.bfloat16)

    # ---- host-side sharding: sort by (node-tile, src-half, relation) ----
    hi = (src >= SPLIT).astype(np.int64)
    core = tgt // NPC
    local = tgt - core * NPC
    ntile = local // M
    ntg = core * NT + ntile
    # src as final key: gather reads walk HBM monotonically within each run
    order = np.lexsort((src, et, hi, ntg))
    src_s = src[order]
    et_s = et[order]
    ew_s = ew[order]
    hi_s = hi[order]
    ntg_s = ntg[order]
    tgtf_s = (local[order] - (ntg_s % NT) * M).astype(np.float32)

    # per (core, nt, half, rel) counts -> uniform caps (max over core, nt)
    cell = (ntg_s * 2 + hi_s) * R + et_s  # monotone under the sort
    counts = np.bincount(cell, minlength=NCORES * NT * 2 * R)
    cnt4 = counts.reshape(NCORES * NT, 2, R)
    caps_lo = [int(np.ceil(cnt4[:, 0, r].max() / P)) for r in range(R)]
    caps_hi = [int(np.ceil(cnt4[:, 1, r].max() / P)) for r in range(R)]
    T_LO = sum(caps_lo)
    T_HI = sum(caps_hi)
    T = T_LO + T_HI

    # slot base of each (half, rel) run within a node tile
    run_base = np.zeros((2, R), dtype=np.int64)
    off = 0
    for r in range(R):
        run_base[0, r] = off
        off += caps_lo[r] * P
    for r in range(R):
        run_base[1, r] = off
        off += caps_hi[r] * P
    slots_per_nt = off  # == T * P

    starts = np.zeros(NCORES * NT * 2 * R + 1, dtype=np.int64)
    np.cumsum(counts, out=starts[1:])
    pos = np.arange(E, dtype=np.int64) - starts[cell]
    slot = ntg_s * slots_per_nt + run_base[hi_s, et_s] + pos

    nslots = NCORES * NT * slots_per_nt
    idx_flat = np.zeros(nslots, dtype=np.int16)
    tg_flat = np.zeros(nslots, dtype=np.float32)
    ew_flat = np.zeros(nslots, dtype=np.float32)
    idx_flat[slot] = (src_s - hi_s * SPLIT).astype(np.int16)
    tg_flat[slot] = tgtf_s
    ew_flat[slot] = ew_s

    # pad slots repeat the last real index of their (nt, half) section so the
    # gather's HBM reads stay monotone / page-local (ew=0 kills their output)
    real_pos = np.full(nslots, -1, dtype=np.int64)
    real_pos[slot] = slot
    np.maximum.accumulate(real_pos, out=real_pos)
    inslot = np.arange(nslots, dtype=np.int64) % slots_per_nt
    sec_start = (np.arange(nslots, dtype=np.int64) - inslot) + np.where(
        inslot < T_LO * P, 0, T_LO * P
    )
    fill_ok = real_pos >= sec_start
    idx_flat = np.where(fill_ok, idx_flat[np.maximum(real_pos, 0)], idx_flat)

    # dma_gather wrapped index layout per node tile: lo call then hi call
    idx16 = np.empty((NCORES, NT, P, T * 8), dtype=np.int16)
    idx_nt = idx_flat.reshape(NCORES, NT, slots_per_nt)
    for c in range(NCORES):
        for nt in range(NT):
            if T_LO:
                idx16[c, nt, :, :T_LO * 8] = _wrap16(idx_nt[c, nt, :T_LO * P])
            if T_HI:
                idx16[c, nt, :, T_LO * 8:] = _wrap16(idx_nt[c, nt, T_LO * P:])

    # meta_f: (C, NT, P, 4T) with [p, 4t..4t+3] = (tgt, -tgt, -ew, ew)
    tgr = tg_flat.reshape(NCORES, NT, T, P)
    ewr = ew_flat.reshape(NCORES, NT, T, P)
    mf5 = np.stack([tgr, ewr], axis=-1)  # (C, NT, T, P, 2)
    meta_f = np.ascontiguousarray(mf5.transpose(0, 1, 3, 2, 4)).reshape(
        NCORES, NT, P, 2 * T
    )

    iota_arr = np.ascontiguousarray(
        np.broadcast_to(np.arange(M, dtype=ml_dtypes.bfloat16), (P, M))
    )

    key = (N, D, R, NPC, NT, tuple(caps_lo), tuple(caps_hi))
    if key not in _PROG_CACHE:
        _PROG_CACHE[key] = _build_program(N, D, R, NPC, NT, caps_lo, caps_hi)
    nc = _PROG_CACHE[key]

    in_maps = [
        dict(
            x=x,
            wr16=wr,
            iota=iota_arr,
            idx16=idx16[c],
            meta_f=meta_f[c],
        )
        for c in range(NCORES)
    ]
    res = run_bass_kernel_spmd(nc, in_maps, list(range(NCORES)), trace=TRACE)
    LAST_PROFILE = res
    out = np.concatenate([res.results[c]["out"] for c in range(NCORES)], axis=0)
    return out
